# revision 10
# baseline (speedup 1.0000x reference)
"""Trainium2 Bass kernel for nn_BmmEnsemble (ANI-style per-species ensemble MLP).

Math (see module reference): for each species s (4) and ensemble member e (8),
the species' atoms' AEV rows go through a 384->160->128->96->1 MLP with
CELU(0.1) after the first three layers; the output is the global sum over all
atoms of the ensemble-mean of the final scalar.

v2: fp8 DoubleRow pipeline.  celu(z,a) = a*elu(z/a), so the network is
rescaled so every activation is elu (alpha=1) exactly; layers 0/1 store
g = elu(u)+1 >= 0 and fold the "-1" into the next layer's bias.

All three matmul layers run as fp8 (e4m3) DoubleRow matmuls (2 fp8 K-planes
per instruction at 0.5 PE-cycles/output-column = 4x bf16 throughput):

 - L0 (K=384): x is quantized to e4m3 on the host (plus an x/64 copy).
   Weights are hi + 64*lo e4m3 pairs (residual compensation kills the
   systematic weight-quantization bias: measured 5.8e-3 -> 7.7e-5).
   3 DoubleRows per 128-col output tile: planes (k0h,k1h),(k2h,k0lo),
   (k1lo,k2lo).
 - L1 (K=160): per member one hi-e4m3 DoubleRow with planes
   (w1[0:128], g0a) and (zero-padded w1[128:160] tail, merged g0b), plus
   one e5m2 lo-residual DoubleRow on the same ifmaps (9.1e-3 -> 1.3e-3).
 - L2 (K=128): merged-bank layout (4 members' 96 outputs packed into
   3x128 psum rows per quad); each normal bank is ONE DoubleRow with
   planes (piece0, g1[m0]) and (piece1, g1[m1]).  Plain e4m3 (1.9e-4).

Elementwise is split across three engines (PSUM has one DVE read port, so
every psum pass runs at 1 elem/lane/cycle; the split is the only lever):
 - L0 and most L1 activations: single-pass custom-DVE poly
       elu(u)+1 ~= max(u + 1, clamp(1 + k*u, 0, 1)^4)       (k = 0.21)
   writing e4m3 directly.
 - SCAL1 members' L1 and ALL of L2: EXACT elu+1 = r + m split as
   rho = Relu(-u-b) (ScalarE, psum), m = Exp(-rho) (ScalarE, sbuf),
   r = max(u+b, 0) (GPSIMD scalar_tensor_tensor, psum).  The consuming
   DoubleRow takes (r, m) as its two planes with the same weights, so
   r+m never needs an add pass.  For L2 the m and r passes write their
   row-sums through the hardware accumulators (accum_out) directly into
   the rs_m / rs_r outputs - no DVE pass at all, and L2 is exact.

Distribution: data-parallel over atoms (2048 atoms/species/core), per-species
weights replicated, host applies the tiny w3 dot and sums the per-core
row-sum outputs (the "all-reduce").  Expected end-to-end error ~3e-4 from
host emulation (gate 2e-2).
"""

import os

import numpy as np

import concourse.dve_ops as _dve_ops
import concourse.mybir as mybir
import concourse.tile as tile
from concourse import bacc
from concourse.bass_utils import run_bass_kernel_spmd
from operator import add as _operator_add

from concourse.dve_spec import (
    C0,
    C1,
    C2,
    One,
    Spec,
    Src0,
    Src1,
    Zero,
    _has_src1,
    lower,
    maxx,
    minn,
    relu,
    sq,
)
from concourse.dve_uop import DveOpSpec

# ---------------------------------------------------------------- constants
S, E = 4, 8
N_ATOMS = 65536
N_CORES = 8
A_SP = N_ATOMS // S // N_CORES      # atoms per species per core = 2048
CHUNK = 512
NCH = A_SP // CHUNK                 # 4 chunks
K0, H0, H1, H2 = 384, 160, 128, 96
KT = K0 // 128                      # 3 K-tiles for layer 0
NP0 = 2                             # DoubleRow pairs for layer 0
NQ = 2                              # member quads per species (E/4)
ALPHA = 0.1
KP = 0.21                           # (1 + KP*u)^4 ~ e^u
LO_SCALE = 64.0                     # w0 residual scale
SCAL1 = ()                          # members-within-quad on the exact L1 path
SCALA_BANKS = (0, 1, 2)             # L2 banks on the ScalarE 3-pass accum path

F32 = mybir.dt.float32
F32R = mybir.dt.float32r
BF16 = mybir.dt.bfloat16
F8 = mybir.dt.float8e4
F8E5 = mybir.dt.float8e5
DR = mybir.MatmulPerfMode.DoubleRow
EXP = mybir.ActivationFunctionType.Exp
RELU = mybir.ActivationFunctionType.Relu
ADD = mybir.AluOpType.add
MUL = mybir.AluOpType.mult
MAX = mybir.AluOpType.max

# ------------------------------------------------------- custom DVE op
# POLY_ELU4: out = max(z + C1, clamp(z*C2 + C0, 0, 1)^4)  ==  elu(u)+1 approx
# with u = z + b;  C0 = 1 + k*b (per-partition), C1 = b + 1, C2 = k.
_B_POLY = maxx(Src0 + C1, sq(sq(minn(relu(Src0 * C2 + C0), One))))
# CELU blend (exact, alpha=1): in1 = exp(u) from ScalarE;
# out = relu(z + C0) + min(in1*C1 - C1, 0) = elu(u) for C1 = 1.
_B_BLEND = relu(Src0 + C0) + minn(Src1 * C1 - C1, Zero)


def _ref_poly(in0, in1, s0, s1, imm2):
    z = in0.astype(np.float32)
    s = np.minimum(np.maximum(z * imm2 + s0, 0.0), 1.0)
    return np.maximum(z + s1, (s * s) * (s * s)).astype(np.float32)


def _ref_blend_acc(in0, in1, s0, s1, imm2):
    z = in0.astype(np.float32) + s0
    b = (np.maximum(z, 0.0)
         + np.minimum(in1.astype(np.float32) * s1 - s1, 0.0)).astype(np.float32)
    return b, b.reshape(b.shape[0], -1).sum(axis=-1, keepdims=True)


def _mk_op(name, spec):
    row = _dve_ops._CUSTOM_DVE_ROW_BASE + len(_dve_ops.OPS)
    assert row < 0x20, "custom-DVE opcode rows exhausted"
    _dve_ops._SUB_OPCODE_FOR_NAME[name] = row
    shas = {}
    for ver in ("v3", "v4"):
        s = DveOpSpec(
            name=name, opcode=row, uops=lower(spec, ver=ver), rd1_en=_has_src1(spec)
        )
        shas[ver] = s.sha(ver)
    op = _dve_ops.DveOp(name, spec, subdim=False, uops_sha=shas)
    _dve_ops.OPS.append(op)
    _dve_ops.CUSTOM_DVE_SPECS[name] = spec
    return op


def _register_ops():
    existing = {o.name: o for o in _dve_ops.OPS}
    if "POLY_ELU4_ANT" in existing:
        return existing["POLY_ELU4_ANT"], existing["ELU_BLEND_ACC_ANT"]
    poly = _mk_op("POLY_ELU4_ANT", Spec(body=_B_POLY, reference=_ref_poly))
    blend = _mk_op(
        "ELU_BLEND_ACC_ANT",
        Spec(body=_B_BLEND, accum=_operator_add, accum_init=Zero,
             reference=_ref_blend_acc),
    )
    return poly, blend


# ------------------------------------------------------------ device build
_NC = None

# merged-L2 bank layout: per quad, (bank, piece) -> (member_in_quad,
# w2-col range, psum-row offset)
_L2_PIECES = [
    [(0, 0, 96, 0), (1, 0, 32, 96)],
    [(1, 32, 96, 0), (2, 0, 64, 64)],
    [(2, 64, 96, 0), (3, 0, 96, 32)],
]

# G1 slot layout within a quad: scal members own (r, m) slot pairs, poly
# members one slot.
_G1_SLOT = {}
_sl = 0
for _m in range(4):
    _G1_SLOT[_m] = _sl
    _sl += 2 if _m in SCAL1 else 1
G1_NSLOT = _sl


def _build_nc():
    global _NC
    if _NC is not None:
        return _NC
    POLY, BLEND_ACC = _register_ops()

    nc = bacc.Bacc("TRN2", target_bir_lowering=False, debug=False)

    # per-core inputs: fp8 feature-major atoms in DoubleRow plane pairs
    xq_d = nc.dram_tensor("xq", [S, 128, NP0, 2, A_SP], F8, kind="ExternalInput")
    # replicated weight packs
    w0a_d = nc.dram_tensor("w0a", [S, 128, NP0, 2, E * 128], F8, kind="ExternalInput")
    w0b_d = nc.dram_tensor("w0b", [S, 128, NP0, 2, NQ * 128], F8, kind="ExternalInput")
    w1h_d = nc.dram_tensor("w1h", [S, 128, 2, E * 128], F8, kind="ExternalInput")
    w1l_d = nc.dram_tensor("w1l", [S, 128, 2, E * 128], F8E5, kind="ExternalInput")
    w2p_d = nc.dram_tensor("w2p", [S, 128, 2, NQ * 3 * 128], F8, kind="ExternalInput")
    # bias packs; *_c0 = 1 + k*b (poly clamp offset), *_c1 = b + 1 (linear);
    # *_b / *_nb = plain / negated bias for the exact (scal) path.
    b0a_c0 = nc.dram_tensor("b0a_c0", [128, S * E], F32, kind="ExternalInput")
    b0a_c1 = nc.dram_tensor("b0a_c1", [128, S * E], F32, kind="ExternalInput")
    b0b_c0 = nc.dram_tensor("b0b_c0", [128, S * NQ], F32, kind="ExternalInput")
    b0b_c1 = nc.dram_tensor("b0b_c1", [128, S * NQ], F32, kind="ExternalInput")
    b1_c0 = nc.dram_tensor("b1_c0", [H1, S * E], F32, kind="ExternalInput")
    b1_c1 = nc.dram_tensor("b1_c1", [H1, S * E], F32, kind="ExternalInput")
    b1_b = nc.dram_tensor("b1_b", [H1, S * E], F32, kind="ExternalInput")
    b1_nb = nc.dram_tensor("b1_nb", [H1, S * E], F32, kind="ExternalInput")
    b2_d = nc.dram_tensor("b2_d", [128, S * NQ * 3], F32, kind="ExternalInput")
    b2_nb = nc.dram_tensor("b2_nb", [128, S * NQ * 3], F32, kind="ExternalInput")
    # outputs: blend banks write row-sums of elu(u2) into rs; scalA banks
    # write row-sums of r2 and m2 (elu+1 = r2+m2) into rs_r / rs_m.
    rs_d = nc.dram_tensor("rs", [128, S * NQ * 3 * NCH], F32, kind="ExternalOutput")
    rsr_d = nc.dram_tensor("rs_r", [128, S * NQ * 3 * NCH], F32, kind="ExternalOutput")
    rsm_d = nc.dram_tensor("rs_m", [128, S * NQ * 3 * NCH], F32, kind="ExternalOutput")

    with tile.TileContext(nc) as tc:
        with (
            tc.tile_pool(name="xp", bufs=2) as xp,
            tc.tile_pool(name="wp", bufs=2) as wp,
            tc.tile_pool(name="bp", bufs=1) as bp,
            tc.tile_pool(name="gp", bufs=2) as gp,
            tc.tile_pool(name="ep", bufs=3) as ep,
            tc.tile_pool(name="ps", bufs=2, space="PSUM") as psp,
        ):
            # warm the ACT Exp/Relu table during the initial DMA wait
            warm = bp.tile([1, 1], F32, tag="warm", name="warm")
            nc.vector.memset(warm[:], 0.0)
            nc.scalar.activation(warm[:], warm[:], EXP)


            B = {}
            _bias_dmas = []
            for nm, d, p in (
                ("b0a_c0", b0a_c0, 128), ("b0a_c1", b0a_c1, 128),
                ("b0b_c0", b0b_c0, 128), ("b0b_c1", b0b_c1, 128),
                ("b1_c0", b1_c0, H1), ("b1_c1", b1_c1, H1),
                ("b1_b", b1_b, H1), ("b1_nb", b1_nb, H1),
                ("b2_d", b2_d, 128), ("b2_nb", b2_nb, 128),
            ):
                t = bp.tile([p, d.shape[-1]], F32, tag=nm, name=nm)
                _bias_dmas.append((t, d))
                B[nm] = t
            RS = bp.tile([128, S * NQ * 3 * NCH], F32, tag="RS", name="RS")
            RSR = bp.tile([128, S * NQ * 3 * NCH], F32, tag="RSR", name="RSR")
            RSM = bp.tile([128, S * NQ * 3 * NCH], F32, tag="RSM", name="RSM")
            nc.vector.memset(RS[:], 0.0)
            nc.vector.memset(RSR[:], 0.0)
            nc.vector.memset(RSM[:], 0.0)

            for s in range(S):
                xt = xp.tile([128, NP0, 2, A_SP], F8, tag="x", name=f"x_{s}")
                # first-chunk x + all weights first so chunk-0 compute starts
                # as early as possible; remaining x chunks stream after.
                nc.sync.dma_start(xt[:, :, :, 0:CHUNK], xq_d[s, :, :, :, 0:CHUNK])
                w0at = wp.tile([128, NP0, 2, E * 128], F8, tag="w0a", name=f"w0a_{s}")
                nc.sync.dma_start(w0at[:], w0a_d[s])
                w0bt = wp.tile([128, NP0, 2, NQ * 128], F8, tag="w0b", name=f"w0b_{s}")
                nc.sync.dma_start(w0bt[:], w0b_d[s])
                if s == 0:
                    for t, d in _bias_dmas:
                        nc.sync.dma_start(t[:], d[:])
                w1ht = wp.tile([128, 2, E * 128], F8, tag="w1h", name=f"w1h_{s}")
                nc.sync.dma_start(w1ht[:], w1h_d[s])
                w1lt = wp.tile([128, 2, E * 128], F8E5, tag="w1l", name=f"w1l_{s}")
                nc.sync.dma_start(w1lt[:], w1l_d[s])
                w2pt = wp.tile([128, 2, NQ * 3 * 128], F8, tag="w2p", name=f"w2p_{s}")
                nc.sync.dma_start(w2pt[:], w2p_d[s])
                nc.sync.dma_start(
                    xt[:, :, :, CHUNK:A_SP], xq_d[s, :, :, :, CHUNK:A_SP]
                )

                for c in range(NCH):
                    cs = slice(c * CHUNK, (c + 1) * CHUNK)
                    for q in range(NQ):
                        sq_i = s * NQ + q
                        # ---- merged layer-0b for the 4 members of this quad
                        ps0b = psp.tile([128, CHUNK], F32, tag="l0b", bufs=2)
                        for p in range(NP0):
                            nc.tensor.matmul(
                                ps0b[:],
                                w0bt[:, p, :, q * 128:(q + 1) * 128],
                                xt[:, p, :, cs],
                                start=(p == 0),
                                stop=(p == NP0 - 1),
                                perf_mode=DR,
                            )
                        # G0: slots 0-3 = member g0a, slot 4 = merged g0b
                        G0 = gp.tile([128, 5, CHUNK], F8, tag="G0",
                                     name=f"G0_{s}_{c}_{q}")
                        nc.vector._custom_dve(
                            POLY, out=G0[:, 4, :], in0=ps0b[:],
                            s0=B["b0b_c0"][:, sq_i:sq_i + 1],
                            s1=B["b0b_c1"][:, sq_i:sq_i + 1], imm2=KP,
                        )
                        # G1: scal members own (r, m) slot pairs, poly one slot
                        G1 = gp.tile([128, G1_NSLOT, CHUNK], F8, tag="G1",
                                     name=f"G1_{s}_{c}_{q}")

                        def do_l2_bank(b):
                            (m0, _, _, _), (m1, _, _, _) = _L2_PIECES[b]
                            ps2 = psp.tile([128, CHUNK], F32, tag="l2",
                                           name=f"ps2_{b}", bufs=2)
                            off = (q * 3 + b) * 128
                            for piece, m in ((0, m0), (1, m1)):
                                sl = _G1_SLOT[m]
                                if m in SCAL1:
                                    # (r, m) planes, same weights
                                    nc.tensor.matmul(
                                        ps2[:],
                                        w2pt[:, piece:piece + 1, off:off + 128]
                                        .broadcast_to([128, 2, 128]),
                                        G1[:, sl:sl + 2, :],
                                        start=(piece == 0), stop=(piece == 1),
                                        perf_mode=DR,
                                    )
                                else:
                                    nc.tensor.matmul(
                                        ps2[:],
                                        w2pt[:, piece, off:off + 128],
                                        G1[:, sl, :],
                                        start=(piece == 0), stop=(piece == 1),
                                    )
                            sqb = (s * NQ + q) * 3 + b
                            col = sqb * NCH + c
                            if b in SCALA_BANKS:
                                # exact elu+1 = r2 + m2, pure ScalarE with
                                # hw-accumulated row-sums.
                                rho2 = ep.tile([128, CHUNK], BF16, tag="rho2",
                                               name=f"rho2_{b}")
                                nc.scalar.activation(
                                    rho2[:], ps2[:], RELU,
                                    bias=B["b2_nb"][:, sqb:sqb + 1], scale=-1.0,
                                )
                                scrm = ep.tile([128, CHUNK], BF16, tag="scrm",
                                               name=f"scrm_{b}")
                                nc.scalar.activation(
                                    scrm[:], rho2[:], EXP, scale=-1.0,
                                    accum_out=RSM[:, col:col + 1],
                                )
                                scrr = ep.tile([128, CHUNK], BF16, tag="scrr",
                                               name=f"scrr_{b}")
                                nc.scalar.activation(
                                    scrr[:], ps2[:], RELU,
                                    bias=B["b2_d"][:, sqb:sqb + 1], scale=1.0,
                                    accum_out=RSR[:, col:col + 1],
                                )
                            else:
                                # v1-style exact blend: ScalarE Exp + DVE
                                # blend with fused row-sum accum (-> elu).
                                e2 = ep.tile([128, CHUNK], F32, tag="e2",
                                             name=f"e2_{b}")
                                nc.scalar.activation(
                                    e2[:], ps2[:], EXP,
                                    bias=B["b2_d"][:, sqb:sqb + 1], scale=1.0,
                                )
                                scr = ep.tile([128, CHUNK], F32R, tag="scr",
                                              name=f"scr_{b}")
                                nc.vector._custom_dve(
                                    BLEND_ACC, out=scr[:],
                                    accum_out=RS[:, col:col + 1],
                                    in0=ps2[:], in1=e2[:],
                                    s0=B["b2_d"][:, sqb:sqb + 1], s1=1.0,
                                )

                        n_done = 0
                        for e in range(q * 4, q * 4 + 4):
                            se = s * E + e
                            e4 = e % 4
                            # ---- layer 0a (first 128 features of member e)
                            ps0a = psp.tile([128, CHUNK], F32, tag="l0a", bufs=2)
                            for p in range(NP0):
                                nc.tensor.matmul(
                                    ps0a[:],
                                    w0at[:, p, :, e * 128:(e + 1) * 128],
                                    xt[:, p, :, cs],
                                    start=(p == 0),
                                    stop=(p == NP0 - 1),
                                    perf_mode=DR,
                                )
                            nc.vector._custom_dve(
                                POLY, out=G0[:, e4, :], in0=ps0a[:],
                                s0=B["b0a_c0"][:, se:se + 1],
                                s1=B["b0a_c1"][:, se:se + 1], imm2=KP,
                            )
                            # ---- layer 1: one hi DR + one e5m2 lo DR on the
                            # (g0a, g0b) plane pair
                            ps1 = psp.tile([H1, CHUNK], F32, tag="l1", bufs=2)
                            ifm = G0[:, e4:5:4 - e4, :]     # planes (e4, 4)
                            nc.tensor.matmul(
                                ps1[:], w1ht[:, :, e * 128:(e + 1) * 128], ifm,
                                start=True, stop=False, perf_mode=DR,
                            )
                            nc.tensor.matmul(
                                ps1[:], w1lt[:, :, e * 128:(e + 1) * 128], ifm,
                                start=False, stop=True, perf_mode=DR,
                            )
                            sl = _G1_SLOT[e4]
                            if e4 in SCAL1:
                                # exact elu+1 = r + m, pure ScalarE 3-pass
                                rho = ep.tile([H1, CHUNK], BF16, tag="rho",
                                              name=f"rho_{e}")
                                nc.scalar.activation(
                                    rho[:], ps1[:], RELU,
                                    bias=B["b1_nb"][:, se:se + 1], scale=-1.0,
                                )
                                nc.scalar.activation(
                                    G1[:, sl + 1, :], rho[:], EXP, scale=-1.0,
                                )
                                nc.scalar.activation(
                                    G1[:, sl, :], ps1[:], RELU,
                                    bias=B["b1_b"][:, se:se + 1], scale=1.0,
                                )
                            else:
                                nc.vector._custom_dve(
                                    POLY, out=G1[:, sl, :], in0=ps1[:],
                                    s0=B["b1_c0"][:, se:se + 1],
                                    s1=B["b1_c1"][:, se:se + 1], imm2=KP,
                                )
                            n_done += 1
                            if n_done >= 2:
                                do_l2_bank(n_done - 2)
            nc.sync.dma_start(rs_d[:], RS[:])
            nc.sync.dma_start(rsr_d[:], RSR[:])
            nc.sync.dma_start(rsm_d[:], RSM[:])
    nc.compile()
    _NC = nc
    return nc


# ------------------------------------------------------------- host side
def _q8(a):
    import ml_dtypes
    return np.clip(a, -240.0, 240.0).astype(ml_dtypes.float8_e4m3)


def _q5(a):
    import ml_dtypes
    return np.clip(a, -57344.0, 57344.0).astype(ml_dtypes.float8_e5m2)


def _prep_shared(w0, w1, w2, b0, b1, b2):
    """Pack rescaled weights/biases into the fp8 DoubleRow device layouts."""
    f = np.float32
    w0h_f = w0.astype(np.float64) / ALPHA                              # [S,E,384,160]
    b0e = b0[:, :, 0, :].astype(np.float64) / ALPHA                    # [S,E,160]
    b1e = b1[:, :, 0, :].astype(np.float64) / ALPHA - w1.astype(np.float64).sum(2)
    b2e = b2[:, :, 0, :].astype(np.float64) / ALPHA - w2.astype(np.float64).sum(2)

    # --- w0 e4m3 planes: pairs (k0,k1), (k2, zero)
    w0hi = _q8(w0h_f).astype(np.float64)                               # [S,E,384,160]
    w0z = np.zeros_like(w0hi)
    sel = [[(w0hi, 0), (w0hi, 1)], [(w0hi, 2), (w0z, 0)]]
    w0a = np.zeros((S, 128, NP0, 2, E * 128), dtype=f)
    w0b = np.zeros((S, 128, NP0, 2, NQ * 128), dtype=f)
    for p in range(NP0):
        for pl in range(2):
            arr, kt = sel[p][pl]
            blk = arr[:, :, kt * 128:(kt + 1) * 128, :]                # [S,E,128,160]
            w0a[:, :, p, pl, :] = (
                blk[..., :128].transpose(0, 2, 1, 3).reshape(S, 128, E * 128)
            )
            w0b[:, :, p, pl, :] = (
                blk[..., 128:H0].transpose(0, 2, 1, 3).reshape(S, 128, E * 32)
            )
    # --- w1 hi e4m3 + lo e5m2, planes (rows 0:128, padded tail)
    w1_64 = w1.astype(np.float64)
    w1hi_f = _q8(w1_64).astype(np.float64)
    w1lo_f = w1_64 - w1hi_f

    def pack_w1(arr):                                                  # [S,E,160,128]
        out = np.zeros((S, 128, 2, E * H1), dtype=np.float64)
        for e in range(E):
            out[:, :, 0, e * H1:(e + 1) * H1] = arr[:, e, :128, :]
            r0 = (e % 4) * 32
            out[:, r0:r0 + 32, 1, e * H1:(e + 1) * H1] = arr[:, e, 128:160, :]
        return out

    w1h = _q8(pack_w1(w1hi_f))
    w1l = _q5(pack_w1(w1lo_f))
    # --- w2 merged-bank planes, e4m3
    w2_64 = w2.astype(np.float64)
    w2p = np.zeros((S, 128, 2, NQ * 3 * 128), dtype=f)
    b2m = np.zeros((S, NQ, 3, 128), dtype=np.float64)
    for s in range(S):
        for q in range(NQ):
            for b in range(3):
                off = (q * 3 + b) * 128
                for piece, (mi, lo, hi, row) in enumerate(_L2_PIECES[b]):
                    e = 4 * q + mi
                    w2p[s, :, piece, off + row:off + row + hi - lo] = (
                        w2_64[s, e, :, lo:hi]
                    )
                    b2m[s, q, b, row:row + hi - lo] = b2e[s, e, lo:hi]
    w2p = _q8(w2p)

    def col_pack(b, lo, hi):
        return np.ascontiguousarray(b[:, :, lo:hi].reshape(S * E, hi - lo).T)

    b0a = col_pack(b0e, 0, 128)                                        # [128, S*E]
    b0b = np.ascontiguousarray(
        b0e[:, :, 128:].reshape(S, NQ, 4 * 32).transpose(2, 0, 1).reshape(128, S * NQ)
    )
    b1c = col_pack(b1e, 0, H1)                                         # [128, S*E]
    b2c = np.ascontiguousarray(b2m.reshape(S * NQ * 3, 128).T)

    shared = {
        "w0a": _q8(w0a), "w0b": _q8(w0b),
        "w1h": w1h, "w1l": w1l, "w2p": w2p,
        "b1_b": b1c.astype(f), "b1_nb": (-b1c).astype(f),
        "b2_d": b2c.astype(f), "b2_nb": (-b2c).astype(f),
    }
    for nm, b in (("b0a", b0a), ("b0b", b0b), ("b1", b1c)):
        shared[f"{nm}_c0"] = (1.0 + KP * b).astype(f)
        shared[f"{nm}_c1"] = (b + 1.0).astype(f)
    return shared


def _prep_core_x(aev_flat, idx_c):
    x = aev_flat[idx_c.reshape(-1)].reshape(S, A_SP, K0)     # [S,A_SP,384]
    xt = x.transpose(0, 2, 1)                                # [S,384,A_SP]
    xhi = _q8(xt).astype(np.float32)
    xq = np.zeros((S, 128, NP0, 2, A_SP), dtype=np.float32)
    selx = [[(xhi, 0), (xhi, 1)], [(xhi, 2), (xhi, 0)]]
    for p in range(NP0):
        for pl in range(2):
            arr, kt = selx[p][pl]
            xq[:, :, p, pl, :] = arr[:, kt * 128:(kt + 1) * 128, :]
    return _q8(xq)


def _host_tail(results, w3, b3):
    """Blend banks: rs = row-sums of elu(u2).  ScalA banks: rs_r + rs_m =
    row-sums of elu(u2)+1 (subtract CHUNK).  Per-atom E = a*w3 . h2 + b3."""
    w3m = np.zeros((128, S, NQ, 3), dtype=np.float64)
    scala = np.zeros((S, NQ, 3), dtype=bool)
    scala[:, :, list(SCALA_BANKS)] = True
    for s in range(S):
        for q in range(NQ):
            for b in range(3):
                for (mi, lo, hi, row) in _L2_PIECES[b]:
                    w3m[row:row + hi - lo, s, q, b] = w3[s, 4 * q + mi, lo:hi, 0]
    w3rep = np.repeat(
        w3m.reshape(128, S * NQ * 3)[:, :, None], NCH, axis=2
    ).reshape(128, S * NQ * 3 * NCH)
    scala_rep = np.repeat(scala.reshape(S * NQ * 3)[:, None], NCH, axis=1
                          ).reshape(-1)[None, :]
    total = 0.0
    for cc in range(N_CORES):
        h2sum = np.where(
            scala_rep,
            results[cc]["rs_r"].astype(np.float64)
            + results[cc]["rs_m"].astype(np.float64) - CHUNK,
            results[cc]["rs"].astype(np.float64),
        )
        total += ALPHA * float((h2sum * w3rep).sum())
    total += float(b3.astype(np.float64).sum()) * (N_ATOMS // S)
    return np.array([total / E], dtype=np.float32)


def _run(inputs, trace=False, tmpdir=None):
    aev = np.asarray(inputs["aev"], dtype=np.float32)
    idx = np.asarray(inputs["idx"], dtype=np.int32)
    w3 = np.asarray(inputs["w3"], dtype=np.float32)
    b3 = np.asarray(inputs["b3"], dtype=np.float32)

    nc = _build_nc()
    shared = _prep_shared(
        np.asarray(inputs["w0"], dtype=np.float32),
        np.asarray(inputs["w1"], dtype=np.float32),
        np.asarray(inputs["w2"], dtype=np.float32),
        np.asarray(inputs["b0"], dtype=np.float32),
        np.asarray(inputs["b1"], dtype=np.float32),
        np.asarray(inputs["b2"], dtype=np.float32),
    )

    aev_flat = aev.reshape(-1, K0)
    in_maps = []
    for cc in range(N_CORES):
        idx_c = idx[:, cc * A_SP:(cc + 1) * A_SP]                # [S, A_SP]
        in_maps.append({"xq": _prep_core_x(aev_flat, idx_c), **shared})

    res = run_bass_kernel_spmd(
        nc, in_maps, core_ids=list(range(N_CORES)), trace=trace, tmpdir=tmpdir
    )
    out = _host_tail(res.results, w3, b3)
    return out, res


def kernel(**inputs):
    out, _ = _run(inputs, trace=bool(int(os.environ.get("BASS_KERNEL_TRACE", "0"))))
    return out


# revision 11
# speedup vs baseline: 1.1919x; 1.1919x over previous
"""Trainium2 Bass kernel for nn_BmmEnsemble (ANI-style per-species ensemble MLP).

Math (see module reference): for each species s (4) and ensemble member e (8),
the species' atoms' AEV rows go through a 384->160->128->96->1 MLP with
CELU(0.1) after the first three layers; the output is the global sum over all
atoms of the ensemble-mean of the final scalar.

v2: fp8 DoubleRow pipeline.  celu(z,a) = a*elu(z/a), so the network is
rescaled so every activation is elu (alpha=1) exactly; layers 0/1 store
g = elu(u)+1 >= 0 and fold the "-1" into the next layer's bias.

All three matmul layers run as fp8 (e4m3) DoubleRow matmuls (2 fp8 K-planes
per instruction at 0.5 PE-cycles/output-column = 4x bf16 throughput):

 - L0 (K=384): x is quantized to e4m3 on the host (plus an x/64 copy).
   Weights are hi + 64*lo e4m3 pairs (residual compensation kills the
   systematic weight-quantization bias: measured 5.8e-3 -> 7.7e-5).
   3 DoubleRows per 128-col output tile: planes (k0h,k1h),(k2h,k0lo),
   (k1lo,k2lo).
 - L1 (K=160): per member one hi-e4m3 DoubleRow with planes
   (w1[0:128], g0a) and (zero-padded w1[128:160] tail, merged g0b), plus
   one e5m2 lo-residual DoubleRow on the same ifmaps (9.1e-3 -> 1.3e-3).
 - L2 (K=128): merged-bank layout (4 members' 96 outputs packed into
   3x128 psum rows per quad); each normal bank is ONE DoubleRow with
   planes (piece0, g1[m0]) and (piece1, g1[m1]).  Plain e4m3 (1.9e-4).

Elementwise is split across three engines (PSUM has one DVE read port, so
every psum pass runs at 1 elem/lane/cycle; the split is the only lever):
 - L0 and most L1 activations: single-pass custom-DVE poly
       elu(u)+1 ~= max(u + 1, clamp(1 + k*u, 0, 1)^4)       (k = 0.21)
   writing e4m3 directly.
 - SCAL1 members' L1 and ALL of L2: EXACT elu+1 = r + m split as
   rho = Relu(-u-b) (ScalarE, psum), m = Exp(-rho) (ScalarE, sbuf),
   r = max(u+b, 0) (GPSIMD scalar_tensor_tensor, psum).  The consuming
   DoubleRow takes (r, m) as its two planes with the same weights, so
   r+m never needs an add pass.  For L2 the m and r passes write their
   row-sums through the hardware accumulators (accum_out) directly into
   the rs_m / rs_r outputs - no DVE pass at all, and L2 is exact.

Distribution: data-parallel over atoms (2048 atoms/species/core), per-species
weights replicated, host applies the tiny w3 dot and sums the per-core
row-sum outputs (the "all-reduce").  Expected end-to-end error ~3e-4 from
host emulation (gate 2e-2).
"""

import os

import numpy as np

import concourse.dve_ops as _dve_ops
import concourse.mybir as mybir
import concourse.tile as tile
from concourse import bacc
from concourse.bass_utils import run_bass_kernel_spmd
from operator import add as _operator_add

from concourse.dve_spec import (
    C0,
    C1,
    C2,
    One,
    Spec,
    Src0,
    Src1,
    Zero,
    _has_src1,
    lower,
    maxx,
    minn,
    relu,
    sq,
)
from concourse.dve_uop import DveOpSpec

# ---------------------------------------------------------------- constants
S, E = 4, 8
N_ATOMS = 65536
N_CORES = 8
A_SP = N_ATOMS // S // N_CORES      # atoms per species per core = 2048
CHUNK = 512
NCH = A_SP // CHUNK                 # 4 chunks
K0, H0, H1, H2 = 384, 160, 128, 96
KT = K0 // 128                      # 3 K-tiles for layer 0
NP0 = 2                             # DoubleRow pairs for layer 0
NQ = 2                              # member quads per species (E/4)
ALPHA = 0.1
KP = 0.21                           # (1 + KP*u)^4 ~ e^u
LO_SCALE = 64.0                     # w0 residual scale
SCAL1 = ()                          # members-within-quad on the exact L1 path
SCALA_BANKS = (0, 1, 2)             # L2 banks on the ScalarE 3-pass accum path

F32 = mybir.dt.float32
F32R = mybir.dt.float32r
BF16 = mybir.dt.bfloat16
F8 = mybir.dt.float8e4
F8E5 = mybir.dt.float8e5
DR = mybir.MatmulPerfMode.DoubleRow
EXP = mybir.ActivationFunctionType.Exp
RELU = mybir.ActivationFunctionType.Relu
ADD = mybir.AluOpType.add
MUL = mybir.AluOpType.mult
MAX = mybir.AluOpType.max

# ------------------------------------------------------- custom DVE op
# POLY_ELU4: out = max(z + C1, clamp(z*C2 + C0, 0, 1)^4)  ==  elu(u)+1 approx
# with u = z + b;  C0 = 1 + k*b (per-partition), C1 = b + 1, C2 = k.
_B_POLY = maxx(Src0 + C1, sq(sq(minn(relu(Src0 * C2 + C0), One))))
# CELU blend (exact, alpha=1): in1 = exp(u) from ScalarE;
# out = relu(z + C0) + min(in1*C1 - C1, 0) = elu(u) for C1 = 1.
_B_BLEND = relu(Src0 + C0) + minn(Src1 * C1 - C1, Zero)


def _ref_poly(in0, in1, s0, s1, imm2):
    z = in0.astype(np.float32)
    s = np.minimum(np.maximum(z * imm2 + s0, 0.0), 1.0)
    return np.maximum(z + s1, (s * s) * (s * s)).astype(np.float32)


def _ref_blend_acc(in0, in1, s0, s1, imm2):
    z = in0.astype(np.float32) + s0
    b = (np.maximum(z, 0.0)
         + np.minimum(in1.astype(np.float32) * s1 - s1, 0.0)).astype(np.float32)
    return b, b.reshape(b.shape[0], -1).sum(axis=-1, keepdims=True)


def _mk_op(name, spec):
    row = _dve_ops._CUSTOM_DVE_ROW_BASE + len(_dve_ops.OPS)
    assert row < 0x20, "custom-DVE opcode rows exhausted"
    _dve_ops._SUB_OPCODE_FOR_NAME[name] = row
    shas = {}
    for ver in ("v3", "v4"):
        s = DveOpSpec(
            name=name, opcode=row, uops=lower(spec, ver=ver), rd1_en=_has_src1(spec)
        )
        shas[ver] = s.sha(ver)
    op = _dve_ops.DveOp(name, spec, subdim=False, uops_sha=shas)
    _dve_ops.OPS.append(op)
    _dve_ops.CUSTOM_DVE_SPECS[name] = spec
    return op


def _register_ops():
    existing = {o.name: o for o in _dve_ops.OPS}
    if "POLY_ELU4_ANT" in existing:
        return existing["POLY_ELU4_ANT"], existing["ELU_BLEND_ACC_ANT"]
    poly = _mk_op("POLY_ELU4_ANT", Spec(body=_B_POLY, reference=_ref_poly))
    blend = _mk_op(
        "ELU_BLEND_ACC_ANT",
        Spec(body=_B_BLEND, accum=_operator_add, accum_init=Zero,
             reference=_ref_blend_acc),
    )
    return poly, blend


# ------------------------------------------------------------ device build
_NC = None

# merged-L2 bank layout: per quad, (bank, piece) -> (member_in_quad,
# w2-col range, psum-row offset)
_L2_PIECES = [
    [(0, 0, 96, 0), (1, 0, 32, 96)],
    [(1, 32, 96, 0), (2, 0, 64, 64)],
    [(2, 64, 96, 0), (3, 0, 96, 32)],
]

# G1 slot layout within a quad: scal members own (r, m) slot pairs, poly
# members one slot.
_G1_SLOT = {}
_sl = 0
for _m in range(4):
    _G1_SLOT[_m] = _sl
    _sl += 2 if _m in SCAL1 else 1
G1_NSLOT = _sl


def _build_nc():
    global _NC
    if _NC is not None:
        return _NC
    POLY, BLEND_ACC = _register_ops()

    nc = bacc.Bacc("TRN2", target_bir_lowering=False, debug=False)

    # per-core inputs: fp8 feature-major atoms in DoubleRow plane pairs
    xq_d = nc.dram_tensor("xq", [S, 128, NP0, 2, A_SP], F8, kind="ExternalInput")
    # replicated weight packs
    w0a_d = nc.dram_tensor("w0a", [S, 128, NP0, 2, E * 128], F8, kind="ExternalInput")
    w0b_d = nc.dram_tensor("w0b", [S, 128, NP0, 2, NQ * 128], F8, kind="ExternalInput")
    w1h_d = nc.dram_tensor("w1h", [S, 128, 2, E * 128], F8, kind="ExternalInput")
    w1l_d = nc.dram_tensor("w1l", [S, 128, 2, E * 128], F8E5, kind="ExternalInput")
    w2p_d = nc.dram_tensor("w2p", [S, 128, 2, NQ * 3 * 128], F8, kind="ExternalInput")
    # bias packs; *_c0 = 1 + k*b (poly clamp offset), *_c1 = b + 1 (linear);
    # *_b / *_nb = plain / negated bias for the exact (scal) path.
    b0a_c0 = nc.dram_tensor("b0a_c0", [128, S * E], F32, kind="ExternalInput")
    b0a_c1 = nc.dram_tensor("b0a_c1", [128, S * E], F32, kind="ExternalInput")
    b0b_c0 = nc.dram_tensor("b0b_c0", [128, S * NQ], F32, kind="ExternalInput")
    b0b_c1 = nc.dram_tensor("b0b_c1", [128, S * NQ], F32, kind="ExternalInput")
    b1_c0 = nc.dram_tensor("b1_c0", [H1, S * E], F32, kind="ExternalInput")
    b1_c1 = nc.dram_tensor("b1_c1", [H1, S * E], F32, kind="ExternalInput")
    b1_b = nc.dram_tensor("b1_b", [H1, S * E], F32, kind="ExternalInput")
    b1_nb = nc.dram_tensor("b1_nb", [H1, S * E], F32, kind="ExternalInput")
    b2_d = nc.dram_tensor("b2_d", [128, S * NQ * 3], F32, kind="ExternalInput")
    b2_nb = nc.dram_tensor("b2_nb", [128, S * NQ * 3], F32, kind="ExternalInput")
    # outputs: blend banks write row-sums of elu(u2) into rs; scalA banks
    # write row-sums of r2 and m2 (elu+1 = r2+m2) into rs_r / rs_m.
    rs_d = nc.dram_tensor("rs", [128, S * NQ * 3 * NCH], F32, kind="ExternalOutput")
    rsr_d = nc.dram_tensor("rs_r", [128, S * NQ * 3 * NCH], F32, kind="ExternalOutput")
    rsm_d = nc.dram_tensor("rs_m", [128, S * NQ * 3 * NCH], F32, kind="ExternalOutput")

    with tile.TileContext(nc) as tc:
        with (
            tc.tile_pool(name="xp", bufs=2) as xp,
            tc.tile_pool(name="wp", bufs=2) as wp,
            tc.tile_pool(name="bp", bufs=1) as bp,
            tc.tile_pool(name="gp", bufs=2) as gp,
            tc.tile_pool(name="ep", bufs=3) as ep,
            tc.tile_pool(name="ps", bufs=2, space="PSUM") as psp,
        ):
            # warm the ACT Exp/Relu table during the initial DMA wait
            warm = bp.tile([1, 1], F32, tag="warm", name="warm")
            nc.vector.memset(warm[:], 0.0)
            nc.scalar.activation(warm[:], warm[:], EXP)


            B = {}
            _bias_dmas = []
            for nm, d, p in (
                ("b0a_c0", b0a_c0, 128), ("b0a_c1", b0a_c1, 128),
                ("b0b_c0", b0b_c0, 128), ("b0b_c1", b0b_c1, 128),
                ("b1_c0", b1_c0, H1), ("b1_c1", b1_c1, H1),
                ("b1_b", b1_b, H1), ("b1_nb", b1_nb, H1),
                ("b2_d", b2_d, 128), ("b2_nb", b2_nb, 128),
            ):
                t = bp.tile([p, d.shape[-1]], F32, tag=nm, name=nm)
                _bias_dmas.append((t, d))
                B[nm] = t
            RS = bp.tile([128, S * NQ * 3 * NCH], F32, tag="RS", name="RS")
            RSR = bp.tile([128, S * NQ * 3 * NCH], F32, tag="RSR", name="RSR")
            RSM = bp.tile([128, S * NQ * 3 * NCH], F32, tag="RSM", name="RSM")
            nc.vector.memset(RS[:], 0.0)
            nc.vector.memset(RSR[:], 0.0)
            nc.vector.memset(RSM[:], 0.0)

            for s in range(S):
                xt = xp.tile([128, NP0, 2, A_SP], F8, tag="x", name=f"x_{s}")
                # first-chunk x + all weights first so chunk-0 compute starts
                # as early as possible; remaining x chunks stream after.
                nc.sync.dma_start(xt[:, :, :, 0:CHUNK], xq_d[s, :, :, :, 0:CHUNK])
                w0at = wp.tile([128, NP0, 2, E * 128], F8, tag="w0a", name=f"w0a_{s}")
                nc.sync.dma_start(w0at[:], w0a_d[s])
                w0bt = wp.tile([128, NP0, 2, NQ * 128], F8, tag="w0b", name=f"w0b_{s}")
                nc.sync.dma_start(w0bt[:], w0b_d[s])
                if s == 0:
                    for t, d in _bias_dmas:
                        nc.sync.dma_start(t[:], d[:])
                w1ht = wp.tile([128, 2, E * 128], F8, tag="w1h", name=f"w1h_{s}")
                nc.sync.dma_start(w1ht[:], w1h_d[s])
                w1lt = wp.tile([128, 2, E * 128], F8E5, tag="w1l", name=f"w1l_{s}")
                nc.sync.dma_start(w1lt[:], w1l_d[s])
                w2pt = wp.tile([128, 2, NQ * 3 * 128], F8, tag="w2p", name=f"w2p_{s}")
                nc.sync.dma_start(w2pt[:], w2p_d[s])
                nc.sync.dma_start(
                    xt[:, :, :, CHUNK:A_SP], xq_d[s, :, :, :, CHUNK:A_SP]
                )

                for c in range(NCH):
                    cs = slice(c * CHUNK, (c + 1) * CHUNK)
                    for q in range(NQ):
                        sq_i = s * NQ + q
                        # ---- merged layer-0b for the 4 members of this quad
                        ps0b = psp.tile([128, CHUNK], F32, tag="l0b", bufs=1)
                        for p in range(NP0):
                            nc.tensor.matmul(
                                ps0b[:],
                                w0bt[:, p, :, q * 128:(q + 1) * 128],
                                xt[:, p, :, cs],
                                start=(p == 0),
                                stop=(p == NP0 - 1),
                                perf_mode=DR,
                            )
                        # G0: slots 0-3 = member g0a, slot 4 = merged g0b
                        G0 = gp.tile([128, 5, CHUNK], F8, tag="G0",
                                     name=f"G0_{s}_{c}_{q}")
                        nc.vector._custom_dve(
                            POLY, out=G0[:, 4, :], in0=ps0b[:],
                            s0=B["b0b_c0"][:, sq_i:sq_i + 1],
                            s1=B["b0b_c1"][:, sq_i:sq_i + 1], imm2=KP,
                        )
                        # G1: scal members own (r, m) slot pairs, poly one slot
                        G1 = gp.tile([128, G1_NSLOT, CHUNK], F8, tag="G1",
                                     name=f"G1_{s}_{c}_{q}")

                        def do_l2_bank(b):
                            (m0, _, _, _), (m1, _, _, _) = _L2_PIECES[b]
                            ps2 = psp.tile([128, CHUNK], F32, tag="l2",
                                           name=f"ps2_{b}", bufs=2)
                            off = (q * 3 + b) * 128
                            for piece, m in ((0, m0), (1, m1)):
                                sl = _G1_SLOT[m]
                                if m in SCAL1:
                                    # (r, m) planes, same weights
                                    nc.tensor.matmul(
                                        ps2[:],
                                        w2pt[:, piece:piece + 1, off:off + 128]
                                        .broadcast_to([128, 2, 128]),
                                        G1[:, sl:sl + 2, :],
                                        start=(piece == 0), stop=(piece == 1),
                                        perf_mode=DR,
                                    )
                                else:
                                    nc.tensor.matmul(
                                        ps2[:],
                                        w2pt[:, piece, off:off + 128],
                                        G1[:, sl, :],
                                        start=(piece == 0), stop=(piece == 1),
                                    )
                            sqb = (s * NQ + q) * 3 + b
                            col = sqb * NCH + c
                            if b in SCALA_BANKS:
                                # exact elu+1 = r2 + m2, pure ScalarE with
                                # hw-accumulated row-sums.
                                rho2 = ep.tile([128, CHUNK], BF16, tag="rho2",
                                               name=f"rho2_{b}")
                                nc.scalar.activation(
                                    rho2[:], ps2[:], RELU,
                                    bias=B["b2_nb"][:, sqb:sqb + 1], scale=-1.0,
                                )
                                scrm = ep.tile([128, CHUNK], BF16, tag="scrm",
                                               name=f"scrm_{b}")
                                nc.scalar.activation(
                                    scrm[:], rho2[:], EXP, scale=-1.0,
                                    accum_out=RSM[:, col:col + 1],
                                )
                                scrr = ep.tile([128, CHUNK], BF16, tag="scrr",
                                               name=f"scrr_{b}")
                                nc.scalar.activation(
                                    scrr[:], ps2[:], RELU,
                                    bias=B["b2_d"][:, sqb:sqb + 1], scale=1.0,
                                    accum_out=RSR[:, col:col + 1],
                                )
                            else:
                                # v1-style exact blend: ScalarE Exp + DVE
                                # blend with fused row-sum accum (-> elu).
                                e2 = ep.tile([128, CHUNK], F32, tag="e2",
                                             name=f"e2_{b}")
                                nc.scalar.activation(
                                    e2[:], ps2[:], EXP,
                                    bias=B["b2_d"][:, sqb:sqb + 1], scale=1.0,
                                )
                                scr = ep.tile([128, CHUNK], F32R, tag="scr",
                                              name=f"scr_{b}")
                                nc.vector._custom_dve(
                                    BLEND_ACC, out=scr[:],
                                    accum_out=RS[:, col:col + 1],
                                    in0=ps2[:], in1=e2[:],
                                    s0=B["b2_d"][:, sqb:sqb + 1], s1=1.0,
                                )

                        n_done = 0
                        for e in range(q * 4, q * 4 + 4):
                            se = s * E + e
                            e4 = e % 4
                            # ---- layer 0a (first 128 features of member e)
                            ps0a = psp.tile([128, CHUNK], F32, tag="l0a", bufs=3)
                            for p in range(NP0):
                                nc.tensor.matmul(
                                    ps0a[:],
                                    w0at[:, p, :, e * 128:(e + 1) * 128],
                                    xt[:, p, :, cs],
                                    start=(p == 0),
                                    stop=(p == NP0 - 1),
                                    perf_mode=DR,
                                )
                            nc.vector._custom_dve(
                                POLY, out=G0[:, e4, :], in0=ps0a[:],
                                s0=B["b0a_c0"][:, se:se + 1],
                                s1=B["b0a_c1"][:, se:se + 1], imm2=KP,
                            )
                            # ---- layer 1: one hi DR + one e5m2 lo DR on the
                            # (g0a, g0b) plane pair
                            ps1 = psp.tile([H1, CHUNK], F32, tag="l1", bufs=2)
                            ifm = G0[:, e4:5:4 - e4, :]     # planes (e4, 4)
                            nc.tensor.matmul(
                                ps1[:], w1ht[:, :, e * 128:(e + 1) * 128], ifm,
                                start=True, stop=False, perf_mode=DR,
                            )
                            nc.tensor.matmul(
                                ps1[:], w1lt[:, :, e * 128:(e + 1) * 128], ifm,
                                start=False, stop=True, perf_mode=DR,
                            )
                            sl = _G1_SLOT[e4]
                            if e4 in SCAL1:
                                # exact elu+1 = r + m, pure ScalarE 3-pass
                                rho = ep.tile([H1, CHUNK], BF16, tag="rho",
                                              name=f"rho_{e}")
                                nc.scalar.activation(
                                    rho[:], ps1[:], RELU,
                                    bias=B["b1_nb"][:, se:se + 1], scale=-1.0,
                                )
                                nc.scalar.activation(
                                    G1[:, sl + 1, :], rho[:], EXP, scale=-1.0,
                                )
                                nc.scalar.activation(
                                    G1[:, sl, :], ps1[:], RELU,
                                    bias=B["b1_b"][:, se:se + 1], scale=1.0,
                                )
                            else:
                                nc.vector._custom_dve(
                                    POLY, out=G1[:, sl, :], in0=ps1[:],
                                    s0=B["b1_c0"][:, se:se + 1],
                                    s1=B["b1_c1"][:, se:se + 1], imm2=KP,
                                )
                            n_done += 1
                            if n_done >= 2:
                                do_l2_bank(n_done - 2)
            nc.sync.dma_start(rs_d[:], RS[:])
            nc.sync.dma_start(rsr_d[:], RSR[:])
            nc.sync.dma_start(rsm_d[:], RSM[:])
    nc.compile()
    _NC = nc
    return nc


# ------------------------------------------------------------- host side
def _q8(a):
    import ml_dtypes
    return np.clip(a, -240.0, 240.0).astype(ml_dtypes.float8_e4m3)


def _q5(a):
    import ml_dtypes
    return np.clip(a, -57344.0, 57344.0).astype(ml_dtypes.float8_e5m2)


def _prep_shared(w0, w1, w2, b0, b1, b2):
    """Pack rescaled weights/biases into the fp8 DoubleRow device layouts."""
    f = np.float32
    w0h_f = w0.astype(np.float64) / ALPHA                              # [S,E,384,160]
    b0e = b0[:, :, 0, :].astype(np.float64) / ALPHA                    # [S,E,160]
    b1e = b1[:, :, 0, :].astype(np.float64) / ALPHA - w1.astype(np.float64).sum(2)
    b2e = b2[:, :, 0, :].astype(np.float64) / ALPHA - w2.astype(np.float64).sum(2)

    # --- w0 e4m3 planes: pairs (k0,k1), (k2, zero)
    w0hi = _q8(w0h_f).astype(np.float64)                               # [S,E,384,160]
    w0z = np.zeros_like(w0hi)
    sel = [[(w0hi, 0), (w0hi, 1)], [(w0hi, 2), (w0z, 0)]]
    w0a = np.zeros((S, 128, NP0, 2, E * 128), dtype=f)
    w0b = np.zeros((S, 128, NP0, 2, NQ * 128), dtype=f)
    for p in range(NP0):
        for pl in range(2):
            arr, kt = sel[p][pl]
            blk = arr[:, :, kt * 128:(kt + 1) * 128, :]                # [S,E,128,160]
            w0a[:, :, p, pl, :] = (
                blk[..., :128].transpose(0, 2, 1, 3).reshape(S, 128, E * 128)
            )
            w0b[:, :, p, pl, :] = (
                blk[..., 128:H0].transpose(0, 2, 1, 3).reshape(S, 128, E * 32)
            )
    # --- w1 hi e4m3 + lo e5m2, planes (rows 0:128, padded tail)
    w1_64 = w1.astype(np.float64)
    w1hi_f = _q8(w1_64).astype(np.float64)
    w1lo_f = w1_64 - w1hi_f

    def pack_w1(arr):                                                  # [S,E,160,128]
        out = np.zeros((S, 128, 2, E * H1), dtype=np.float64)
        for e in range(E):
            out[:, :, 0, e * H1:(e + 1) * H1] = arr[:, e, :128, :]
            r0 = (e % 4) * 32
            out[:, r0:r0 + 32, 1, e * H1:(e + 1) * H1] = arr[:, e, 128:160, :]
        return out

    w1h = _q8(pack_w1(w1hi_f))
    w1l = _q5(pack_w1(w1lo_f))
    # --- w2 merged-bank planes, e4m3
    w2_64 = w2.astype(np.float64)
    w2p = np.zeros((S, 128, 2, NQ * 3 * 128), dtype=f)
    b2m = np.zeros((S, NQ, 3, 128), dtype=np.float64)
    for s in range(S):
        for q in range(NQ):
            for b in range(3):
                off = (q * 3 + b) * 128
                for piece, (mi, lo, hi, row) in enumerate(_L2_PIECES[b]):
                    e = 4 * q + mi
                    w2p[s, :, piece, off + row:off + row + hi - lo] = (
                        w2_64[s, e, :, lo:hi]
                    )
                    b2m[s, q, b, row:row + hi - lo] = b2e[s, e, lo:hi]
    w2p = _q8(w2p)

    def col_pack(b, lo, hi):
        return np.ascontiguousarray(b[:, :, lo:hi].reshape(S * E, hi - lo).T)

    b0a = col_pack(b0e, 0, 128)                                        # [128, S*E]
    b0b = np.ascontiguousarray(
        b0e[:, :, 128:].reshape(S, NQ, 4 * 32).transpose(2, 0, 1).reshape(128, S * NQ)
    )
    b1c = col_pack(b1e, 0, H1)                                         # [128, S*E]
    b2c = np.ascontiguousarray(b2m.reshape(S * NQ * 3, 128).T)

    shared = {
        "w0a": _q8(w0a), "w0b": _q8(w0b),
        "w1h": w1h, "w1l": w1l, "w2p": w2p,
        "b1_b": b1c.astype(f), "b1_nb": (-b1c).astype(f),
        "b2_d": b2c.astype(f), "b2_nb": (-b2c).astype(f),
    }
    for nm, b in (("b0a", b0a), ("b0b", b0b), ("b1", b1c)):
        shared[f"{nm}_c0"] = (1.0 + KP * b).astype(f)
        shared[f"{nm}_c1"] = (b + 1.0).astype(f)
    return shared


def _prep_core_x(aev_flat, idx_c):
    x = aev_flat[idx_c.reshape(-1)].reshape(S, A_SP, K0)     # [S,A_SP,384]
    xt = x.transpose(0, 2, 1)                                # [S,384,A_SP]
    xhi = _q8(xt).astype(np.float32)
    xq = np.zeros((S, 128, NP0, 2, A_SP), dtype=np.float32)
    selx = [[(xhi, 0), (xhi, 1)], [(xhi, 2), (xhi, 0)]]
    for p in range(NP0):
        for pl in range(2):
            arr, kt = selx[p][pl]
            xq[:, :, p, pl, :] = arr[:, kt * 128:(kt + 1) * 128, :]
    return _q8(xq)


def _host_tail(results, w3, b3):
    """Blend banks: rs = row-sums of elu(u2).  ScalA banks: rs_r + rs_m =
    row-sums of elu(u2)+1 (subtract CHUNK).  Per-atom E = a*w3 . h2 + b3."""
    w3m = np.zeros((128, S, NQ, 3), dtype=np.float64)
    scala = np.zeros((S, NQ, 3), dtype=bool)
    scala[:, :, list(SCALA_BANKS)] = True
    for s in range(S):
        for q in range(NQ):
            for b in range(3):
                for (mi, lo, hi, row) in _L2_PIECES[b]:
                    w3m[row:row + hi - lo, s, q, b] = w3[s, 4 * q + mi, lo:hi, 0]
    w3rep = np.repeat(
        w3m.reshape(128, S * NQ * 3)[:, :, None], NCH, axis=2
    ).reshape(128, S * NQ * 3 * NCH)
    scala_rep = np.repeat(scala.reshape(S * NQ * 3)[:, None], NCH, axis=1
                          ).reshape(-1)[None, :]
    total = 0.0
    for cc in range(N_CORES):
        h2sum = np.where(
            scala_rep,
            results[cc]["rs_r"].astype(np.float64)
            + results[cc]["rs_m"].astype(np.float64) - CHUNK,
            results[cc]["rs"].astype(np.float64),
        )
        total += ALPHA * float((h2sum * w3rep).sum())
    total += float(b3.astype(np.float64).sum()) * (N_ATOMS // S)
    return np.array([total / E], dtype=np.float32)


def _run(inputs, trace=False, tmpdir=None):
    aev = np.asarray(inputs["aev"], dtype=np.float32)
    idx = np.asarray(inputs["idx"], dtype=np.int32)
    w3 = np.asarray(inputs["w3"], dtype=np.float32)
    b3 = np.asarray(inputs["b3"], dtype=np.float32)

    nc = _build_nc()
    shared = _prep_shared(
        np.asarray(inputs["w0"], dtype=np.float32),
        np.asarray(inputs["w1"], dtype=np.float32),
        np.asarray(inputs["w2"], dtype=np.float32),
        np.asarray(inputs["b0"], dtype=np.float32),
        np.asarray(inputs["b1"], dtype=np.float32),
        np.asarray(inputs["b2"], dtype=np.float32),
    )

    aev_flat = aev.reshape(-1, K0)
    in_maps = []
    for cc in range(N_CORES):
        idx_c = idx[:, cc * A_SP:(cc + 1) * A_SP]                # [S, A_SP]
        in_maps.append({"xq": _prep_core_x(aev_flat, idx_c), **shared})

    res = run_bass_kernel_spmd(
        nc, in_maps, core_ids=list(range(N_CORES)), trace=trace, tmpdir=tmpdir
    )
    out = _host_tail(res.results, w3, b3)
    return out, res


def kernel(**inputs):
    out, _ = _run(inputs, trace=bool(int(os.environ.get("BASS_KERNEL_TRACE", "0"))))
    return out


# revision 13
# speedup vs baseline: 1.2090x; 1.0143x over previous
"""Trainium2 Bass kernel for nn_BmmEnsemble (ANI-style per-species ensemble MLP).

Math (see reference): for each species s (4) and ensemble member e (8), the
species' atoms' AEV rows go through a 384->160->128->96->1 MLP with CELU(0.1)
after the first three layers; the output energy is the global sum over all
atoms of the ensemble-mean of the final scalar.

Key transformation: celu(z, a) = a*elu(z/a), so the whole network is rescaled
so the activation becomes elu (alpha=1) exactly: W0 <- W0/a, biases <- b/a,
w3 <- a*w3, activations h_hat = h/a.  On top of that, layers 0/1 store
g = elu(u) + 1 (>= 0) and fold the "-1" into the next layer's bias via
b_next <- b_next - colsum(W_next).

Layers 0/1 evaluate elu(u)+1 with a SINGLE elementwise pass per tile, split
across two engines to break the baseline's VectorE bottleneck:

 - most tiles: one custom-DVE pass
       elu(u)+1 ~= max(u + 1, clamp(1 + k*u, 0, 1)^4)       (k = 0.21)
   using (1+k*u)^4 ~= e^u and Bernoulli ((1+ku)^4 >= 1+u on the clamp
   range), so the max IS the exact relu branch for u>=0.  The body is
   exactly 8 DVE ALU stages: fma, relu, minn, sq, sq, add, maxx.
 - half the L0a tiles (SCAL_MEMBERS) use a pure-ScalarE path instead:
   rho = Relu(-u), m = Exp(-rho), r = Relu(u) (three ACT passes, exact:
   elu(u)+1 = r + m), and the layer-1 matmul consumes r and m as two
   accumulating rhs.  This moves ~80us of elementwise work to the
   otherwise-idle ScalarE.

Layer 2 keeps the baseline's exact two-pass form (ScalarE Exp + DVE blend
with fused row-sum accum) with alpha=1 semantics.  Layer-0 matmuls run in
bf16 (x, w0 both bf16 - mixed bf16 x f32r is rejected by the BIR verifier):
same PE rate, but FWL halves LDWEIGHTS time and input DMA.  Layers 1/2 stay
f32r because DVE bf16 writes are slower (RMW).  Measured end-to-end error:
6e-4 relative (gate 2e-2).

Distribution: data-parallel over atoms (2048 atoms/species/core,
feature-major), per-species ensemble weights replicated, host applies the
tiny w3 dot and sums the per-core row-sum outputs (the "all-reduce").

Measured on 8 axon-tunneled trn2 cores: ~266us HW exec (baseline with
2-pass celu on every tile: ~294us).  Engine busy: VectorE ~252us (320
passes), ScalarE ~199us (289 ACT passes), TensorE ~211us streaming.
Tried and reverted: chunk-pair [128,2,512] DVE/ACT batching cut VectorE
busy to 220us but single-buffered PSUM pairs (8-bank limit) added ~100us
of dependency stalls (315-367us spans); fp8 fails accuracy (5e-2).
"""

import os
from operator import add as _operator_add

import numpy as np

import concourse.dve_ops as _dve_ops
import concourse.mybir as mybir
import concourse.tile as tile
from concourse import bacc
from concourse.bass_utils import run_bass_kernel_spmd
from concourse.dve_spec import (
    C0,
    C1,
    C2,
    One,
    Spec,
    Src0,
    Src1,
    Zero,
    _has_src1,
    lower,
    maxx,
    minn,
    relu,
    sq,
)
from concourse.dve_uop import DveOpSpec

# ---------------------------------------------------------------- constants
S, E = 4, 8
N_ATOMS = 65536
N_CORES = 8
A_SP = N_ATOMS // S // N_CORES      # atoms per species per core = 2048
CHUNK = 512
NCH = A_SP // CHUNK                 # 4 chunks
K0, H0, H1, H2 = 384, 160, 128, 96
KT = K0 // 128                      # 3 K-tiles for layer 0
NQ = 2                              # member quads per species (E/4)
ALPHA = 0.1
KP = 0.21                           # (1 + KP*u)^4 ~ e^u  (layers 0/1)

F32 = mybir.dt.float32
F32R = mybir.dt.float32r
BF16 = mybir.dt.bfloat16
EXP = mybir.ActivationFunctionType.Exp
RELU = mybir.ActivationFunctionType.Relu

# L0a tiles of these members use the pure-ScalarE path (rho/m/r ACT passes,
# consumer matmul takes r and m as two accumulating rhs) instead of the DVE
# poly pass — balances VectorE vs the otherwise-idle ScalarE.
SCAL_MEMBERS = (0, 2, 4, 6)

# ------------------------------------------------------- custom DVE ops
# POLY_ELU4: out = max(z + C1, clamp(z*C2 + C0, 0, 1)^4)  ==  elu(u)+1 approx
# with u = z + b;  C0 = 1 + k*b (per-partition), C1 = b + 1, C2 = k.
_B_POLY = maxx(Src0 + C1, sq(sq(minn(relu(Src0 * C2 + C0), One))))
# CELU blend (exact, for layer 2 with alpha=1): in1 = exp(u) from ScalarE;
# out = relu(z + C0) + min(in1*C1 - C1, 0) = elu(u) for C1 = 1.
_B_BLEND = relu(Src0 + C0) + minn(Src1 * C1 - C1, Zero)


def _ref_poly(in0, in1, s0, s1, imm2):
    z = in0.astype(np.float32)
    s = np.minimum(np.maximum(z * imm2 + s0, 0.0), 1.0)
    return np.maximum(z + s1, (s * s) * (s * s)).astype(np.float32)


def _ref_blend_acc(in0, in1, s0, s1, imm2):
    z = in0.astype(np.float32) + s0
    b = (np.maximum(z, 0.0)
         + np.minimum(in1.astype(np.float32) * s1 - s1, 0.0)).astype(np.float32)
    return b, b.reshape(b.shape[0], -1).sum(axis=-1, keepdims=True)


def _mk_op(name, spec):
    row = _dve_ops._CUSTOM_DVE_ROW_BASE + len(_dve_ops.OPS)
    assert row < 0x20, "custom-DVE opcode rows exhausted"
    _dve_ops._SUB_OPCODE_FOR_NAME[name] = row
    shas = {}
    for ver in ("v3", "v4"):
        s = DveOpSpec(
            name=name, opcode=row, uops=lower(spec, ver=ver), rd1_en=_has_src1(spec)
        )
        shas[ver] = s.sha(ver)
    op = _dve_ops.DveOp(name, spec, subdim=False, uops_sha=shas)
    _dve_ops.OPS.append(op)
    _dve_ops.CUSTOM_DVE_SPECS[name] = spec
    return op


def _register_ops():
    existing = {o.name: o for o in _dve_ops.OPS}
    if "POLY_ELU4_ANT" in existing:
        return existing["POLY_ELU4_ANT"], existing["ELU_BLEND_ACC_ANT"]
    poly = _mk_op("POLY_ELU4_ANT", Spec(body=_B_POLY, reference=_ref_poly))
    blend = _mk_op(
        "ELU_BLEND_ACC_ANT",
        Spec(body=_B_BLEND, accum=_operator_add, accum_init=Zero,
             reference=_ref_blend_acc),
    )
    return poly, blend


# ------------------------------------------------------------ device build
_NC = None


def _build_nc():
    global _NC
    if _NC is not None:
        return _NC
    POLY, BLEND_ACC = _register_ops()

    nc = bacc.Bacc("TRN2", target_bir_lowering=False, debug=False)

    F8 = mybir.dt.float8e4
    DRM = mybir.MatmulPerfMode.DoubleRow
    # per-core inputs (fp8 feature-major atoms in DoubleRow plane pairs
    # (k0,k1),(k2,k0); the second pair's plane-1 weights are zero)
    xt_d = nc.dram_tensor("xt", [S, 128, 2, 2, A_SP], F8, kind="ExternalInput")
    # replicated weight packs (fp8, rescaled: w0 includes the 1/alpha).
    w0a_d = nc.dram_tensor("w0a", [S, 128, 2, 2, E * 128], F8, kind="ExternalInput")
    w0b_d = nc.dram_tensor("w0b4", [S, 128, 2, 2, NQ * 128], F8, kind="ExternalInput")
    w1a_d = nc.dram_tensor("w1a", [S, 128, E * H1], F32R, kind="ExternalInput")
    w1b_d = nc.dram_tensor("w1b", [S, 128, E * H1], F32R, kind="ExternalInput")
    w2_d = nc.dram_tensor("w2p", [S, 128, NQ * 6 * 128], F32R, kind="ExternalInput")
    # bias packs; *_c0 = 1 + k*b (poly clamp offset), *_c1 = b + 1 (linear).
    b0a_c0 = nc.dram_tensor("b0a_c0", [128, S * E], F32, kind="ExternalInput")
    b0a_c1 = nc.dram_tensor("b0a_c1", [128, S * E], F32, kind="ExternalInput")
    b0b_c0 = nc.dram_tensor("b0b_c0", [128, S * NQ], F32, kind="ExternalInput")
    b0b_c1 = nc.dram_tensor("b0b_c1", [128, S * NQ], F32, kind="ExternalInput")
    b1_c0 = nc.dram_tensor("b1_c0", [H1, S * E], F32, kind="ExternalInput")
    b1_c1 = nc.dram_tensor("b1_c1", [H1, S * E], F32, kind="ExternalInput")
    # ScalarE-path packs: plain bias and negated bias
    b0a_b = nc.dram_tensor("b0a_b", [128, S * E], F32, kind="ExternalInput")
    b0a_nb = nc.dram_tensor("b0a_nb", [128, S * E], F32, kind="ExternalInput")
    # L2 (exact 2-pass): single bias pack b2 (ACT bias and blend s0)
    b2_d = nc.dram_tensor("b2m_d", [128, S * NQ * 3], F32, kind="ExternalInput")
    # output: per-(s,quad,bank,chunk) row-sums of h2 = elu(u2) (merged rows)
    rs_d = nc.dram_tensor("rs", [128, S * NQ * 3 * NCH], F32, kind="ExternalOutput")

    with tile.TileContext(nc) as tc:
        with (
            tc.tile_pool(name="xp", bufs=2) as xp,
            tc.tile_pool(name="w0pool", bufs=2) as w0p,
            tc.tile_pool(name="w1pool", bufs=2) as w1p,
            tc.tile_pool(name="bp", bufs=1) as bp,
            tc.tile_pool(name="ep", bufs=4) as ep,
            tc.tile_pool(name="gp", bufs=6) as gp,
            tc.tile_pool(name="ps", bufs=2, space="PSUM") as psp,
        ):
            # warm the ACT Exp table during the initial DMA wait
            warm = bp.tile([1, 1], F32, tag="warm", name="warm")
            nc.vector.memset(warm[:], 0.0)
            nc.scalar.activation(warm[:], warm[:], EXP)

            B = {}
            _bias_dmas = []
            for nm, d, p in (
                ("b0a_c0", b0a_c0, 128), ("b0a_c1", b0a_c1, 128),
                ("b0b_c0", b0b_c0, 128), ("b0b_c1", b0b_c1, 128),
                ("b1_c0", b1_c0, H1), ("b1_c1", b1_c1, H1),
                ("b0a_b", b0a_b, 128), ("b0a_nb", b0a_nb, 128),
                ("b2_d", b2_d, 128),
            ):
                t = bp.tile([p, d.shape[-1]], F32, tag=nm, name=nm)
                _bias_dmas.append((t, d))
                B[nm] = t
            RS = bp.tile([128, S * NQ * 3 * NCH], F32, tag="RS", name="RS")

            for s in range(S):
                # first-chunk x slices + all weights first, so chunk-0 compute
                # starts as early as possible; remaining x chunks stream after
                xt = xp.tile([128, 2, 2, A_SP], F8, tag="x", name=f"x_{s}")
                nc.sync.dma_start(xt[:, :, :, 0:CHUNK], xt_d[s, :, :, :, 0:CHUNK])
                w0at = w0p.tile([128, 2, 2, E * 128], F8, tag="w0a", name=f"w0a_{s}")
                nc.sync.dma_start(w0at[:], w0a_d[s])
                w0bt = w0p.tile([128, 2, 2, NQ * 128], F8, tag="w0b", name=f"w0b_{s}")
                nc.sync.dma_start(w0bt[:], w0b_d[s])
                if s == 0:
                    # bias packs are tiny (~130KB) and needed by the very
                    # first poly pass (~8us in) - emit them before the bulky
                    # w1/w2 transfers so the first quad's elementwise work
                    # isn't stalled behind 3.5MB of layer-1/2 weights.
                    for t, d in _bias_dmas:
                        nc.sync.dma_start(t[:], d[:])
                w1at = w1p.tile([128, E * H1], F32R, tag="w1a", name=f"w1a_{s}")
                nc.sync.dma_start(w1at[:], w1a_d[s])
                w1bt = w1p.tile([128, E * H1], F32R, tag="w1b", name=f"w1b_{s}")
                nc.sync.dma_start(w1bt[:], w1b_d[s])
                w2t = w1p.tile([128, NQ * 6 * 128], F32R, tag="w2", name=f"w2_{s}")
                nc.sync.dma_start(w2t[:], w2_d[s])
                nc.sync.dma_start(
                    xt[:, :, :, CHUNK:A_SP], xt_d[s, :, :, :, CHUNK:A_SP]
                )

                for c in range(NCH):
                    cs = slice(c * CHUNK, (c + 1) * CHUNK)
                    for q in range(NQ):
                        sq_i = s * NQ + q
                        # ---- merged layer-0b for the 4 members of this quad
                        ps0b = psp.tile([128, CHUNK], F32, tag="l0b", bufs=1)
                        for p in range(2):
                            nc.tensor.matmul(
                                ps0b[:],
                                w0bt[:, p, :, q * 128 : (q + 1) * 128],
                                xt[:, p, :, cs],
                                start=(p == 0),
                                stop=(p == 1),
                                perf_mode=DRM,
                            )
                        g0b = gp.tile([128, CHUNK], F32R, tag="g0b")
                        nc.vector._custom_dve(
                            POLY, out=g0b[:], in0=ps0b[:],
                            s0=B["b0b_c0"][:, sq_i : sq_i + 1],
                            s1=B["b0b_c1"][:, sq_i : sq_i + 1], imm2=KP,
                        )

                        def do_l2_bank(b):
                            # merged layer 2, bank b of the quad (2 zero-padded
                            # matmuls); exact elu via ScalarE Exp + DVE blend
                            # with fused row-sum accum into RS.
                            (m0, m1) = ((0, 1), (1, 2), (2, 3))[b]
                            ps2 = psp.tile([128, CHUNK], F32, tag="l2", name=f"ps2_{b}")
                            off = (q * 3 + b) * 2 * 128
                            nc.tensor.matmul(
                                ps2[:], w2t[:, off : off + 128], g1s[m0][:],
                                start=True, stop=False,
                            )
                            nc.tensor.matmul(
                                ps2[:], w2t[:, off + 128 : off + 256], g1s[m1][:],
                                start=False, stop=True,
                            )
                            sqb = (s * NQ + q) * 3 + b
                            e2 = ep.tile([128, CHUNK], F32, tag="e2", name=f"e2_{b}")
                            nc.scalar.activation(
                                e2[:], ps2[:], EXP,
                                bias=B["b2_d"][:, sqb : sqb + 1], scale=1.0,
                            )
                            g2 = gp.tile([128, CHUNK], F32, tag="g2", name=f"g2_{b}")
                            nc.vector._custom_dve(
                                BLEND_ACC, out=g2[:],
                                accum_out=RS[:, sqb * NCH + c : sqb * NCH + c + 1],
                                in0=ps2[:], in1=e2[:],
                                s0=B["b2_d"][:, sqb : sqb + 1], s1=1.0,
                            )

                        g1s = []
                        for e in range(q * 4, q * 4 + 4):
                            se = s * E + e
                            # ---- layer 0a (first 128 features of member e)
                            ps0a = psp.tile([128, CHUNK], F32, tag="l0a", bufs=3)
                            for p in range(2):
                                nc.tensor.matmul(
                                    ps0a[:],
                                    w0at[:, p, :, e * 128 : (e + 1) * 128],
                                    xt[:, p, :, cs],
                                    start=(p == 0),
                                    stop=(p == 1),
                                    perf_mode=DRM,
                                )
                            ps1 = psp.tile([H1, CHUNK], F32, tag="l1", bufs=2)
                            if e in SCAL_MEMBERS:
                                # pure-ScalarE path: g0a = r + m exactly
                                # (elu+1 = relu(u) + exp(-relu(-u))); the L1
                                # matmul consumes r and m as two rhs.
                                rho = ep.tile([128, CHUNK], F32, tag="rho")
                                nc.scalar.activation(
                                    rho[:], ps0a[:], RELU,
                                    bias=B["b0a_nb"][:, se : se + 1], scale=-1.0,
                                )
                                m0 = gp.tile([128, CHUNK], F32R, tag="m0")
                                nc.scalar.activation(
                                    m0[:], rho[:], EXP, scale=-1.0,
                                )
                                r0 = gp.tile([128, CHUNK], F32R, tag="r0")
                                nc.scalar.activation(
                                    r0[:], ps0a[:], RELU,
                                    bias=B["b0a_b"][:, se : se + 1], scale=1.0,
                                )
                                nc.tensor.matmul(
                                    ps1[:], w1at[:, e * H1 : (e + 1) * H1], r0[:],
                                    start=True, stop=False,
                                )
                                nc.tensor.matmul(
                                    ps1[:], w1at[:, e * H1 : (e + 1) * H1], m0[:],
                                    start=False, stop=False,
                                )
                            else:
                                g0a = gp.tile([128, CHUNK], F32R, tag="g0a")
                                nc.vector._custom_dve(
                                    POLY, out=g0a[:], in0=ps0a[:],
                                    s0=B["b0a_c0"][:, se : se + 1],
                                    s1=B["b0a_c1"][:, se : se + 1], imm2=KP,
                                )
                                nc.tensor.matmul(
                                    ps1[:], w1at[:, e * H1 : (e + 1) * H1], g0a[:],
                                    start=True, stop=False,
                                )
                            nc.tensor.matmul(
                                ps1[:], w1bt[:, e * H1 : (e + 1) * H1], g0b[:],
                                start=False, stop=True,
                            )
                            g1 = gp.tile([H1, CHUNK], F32R, tag="g1", bufs=6)
                            nc.vector._custom_dve(
                                POLY, out=g1[:], in0=ps1[:],
                                s0=B["b1_c0"][:, se : se + 1],
                                s1=B["b1_c1"][:, se : se + 1], imm2=KP,
                            )
                            g1s.append(g1)
                            if len(g1s) >= 2:
                                do_l2_bank(len(g1s) - 2)
            nc.sync.dma_start(rs_d[:], RS[:])
    nc.compile()
    _NC = nc
    return nc


# ------------------------------------------------------------- host side
# merged-L2 bank layout: per quad, (bank, piece) -> (member_in_quad,
# w2-col range, psum-row offset)
_L2_PIECES = [
    [(0, 0, 96, 0), (1, 0, 32, 96)],
    [(1, 32, 96, 0), (2, 0, 64, 64)],
    [(2, 64, 96, 0), (3, 0, 96, 32)],
]


def _q8(a):
    import ml_dtypes
    return np.clip(a, -240.0, 240.0).astype(ml_dtypes.float8_e4m3)


def _prep_shared(w0, w1, w2, b0, b1, b2):
    """Pack rescaled weights/biases into the device layouts.

    Rescaling: W0/alpha, biases b/alpha; layer-0/1 activations stored as
    g = elu+1, so layer 1/2 effective bias is b/alpha - colsum(W).
    """
    f = np.float32
    w0 = (w0.astype(np.float64) / ALPHA)
    b0e = (b0[:, :, 0, :].astype(np.float64) / ALPHA)                 # [S,E,160]
    b1e = b1[:, :, 0, :].astype(np.float64) / ALPHA - w1.astype(np.float64).sum(2)
    b2e = b2[:, :, 0, :].astype(np.float64) / ALPHA - w2.astype(np.float64).sum(2)

    w0hi = _q8(w0).astype(np.float64)           # rescaled already (w0/alpha)
    w0z = np.zeros_like(w0hi)
    sel = [[(w0hi, 0), (w0hi, 1)], [(w0hi, 2), (w0z, 0)]]
    w0a = np.zeros((S, 128, 2, 2, E * 128), dtype=f)
    w0b4 = np.zeros((S, 128, 2, 2, NQ * 128), dtype=f)
    for p in range(2):
        for pl in range(2):
            arr, kt = sel[p][pl]
            blk = arr.reshape(S, E, KT, 128, H0)[:, :, kt]      # [S,E,128,160]
            w0a[:, :, p, pl, :] = (
                blk[..., :128].transpose(0, 2, 1, 3).reshape(S, 128, E * 128)
            )
            w0b4[:, :, p, pl, :] = (
                blk[..., 128:].transpose(0, 2, 1, 3).reshape(S, 128, E * 32)
            )
    w0a = _q8(w0a)
    w0b4 = _q8(w0b4)
    w1a = np.ascontiguousarray(
        w1[:, :, :128, :].transpose(0, 2, 1, 3).reshape(S, 128, E * H1)
    ).astype(f)
    w1b = np.zeros((S, 4, 32, E, H1), dtype=f)
    for e in range(E):
        w1b[:, e % 4, :, e, :] = w1[:, e, 128:, :]
    w1b = np.ascontiguousarray(w1b.reshape(S, 128, E * H1))
    # merged-L2 packs
    w2pk = np.zeros((S, NQ, 3, 2, 128, 128), dtype=f)  # [s,q,b,piece,K,M]
    b2m = np.zeros((S, NQ, 3, 128), dtype=np.float64)
    for s in range(S):
        for q in range(NQ):
            for b in range(3):
                for piece, (mi, lo, hi, row) in enumerate(_L2_PIECES[b]):
                    e = 4 * q + mi
                    w2pk[s, q, b, piece, :, row : row + hi - lo] = w2[s, e, :, lo:hi]
                    b2m[s, q, b, row : row + hi - lo] = b2e[s, e, lo:hi]
    w2p = np.ascontiguousarray(
        w2pk.transpose(0, 4, 1, 2, 3, 5).reshape(S, 128, NQ * 6 * 128)
    )

    def col_pack(b, lo, hi):
        # b [S,E,P] -> [hi-lo, S*E]
        return np.ascontiguousarray(
            b[:, :, lo:hi].reshape(S * E, hi - lo).T
        ).astype(np.float64)

    b0a_c1 = col_pack(b0e, 0, 128)
    b0b_c1 = np.ascontiguousarray(
        b0e[:, :, 128:].reshape(S, NQ, 4 * (H0 - 128)).transpose(2, 0, 1).reshape(128, S * NQ)
    )
    b1_c1 = col_pack(b1e, 0, H1)
    b2m_d = np.ascontiguousarray(b2m.reshape(S * NQ * 3, 128).T).astype(f)

    shared = {
        "w0a": w0a, "w0b4": w0b4, "w1a": w1a, "w1b": w1b, "w2p": w2p,
        "b2m_d": b2m_d,
    }
    for nm, b in (("b0a", b0a_c1), ("b0b", b0b_c1), ("b1", b1_c1)):
        shared[f"{nm}_c0"] = (1.0 + KP * b).astype(f)
        shared[f"{nm}_c1"] = (b + 1.0).astype(f)
    shared["b0a_b"] = b0a_c1.astype(f)
    shared["b0a_nb"] = (-b0a_c1).astype(f)
    return shared


def _run(inputs, trace=False, tmpdir=None):
    import ml_dtypes

    aev = np.asarray(inputs["aev"], dtype=np.float32)
    idx = np.asarray(inputs["idx"], dtype=np.int32)
    w3 = np.asarray(inputs["w3"], dtype=np.float32)
    b3 = np.asarray(inputs["b3"], dtype=np.float32)

    nc = _build_nc()
    shared = _prep_shared(
        np.asarray(inputs["w0"], dtype=np.float32),
        np.asarray(inputs["w1"], dtype=np.float32),
        np.asarray(inputs["w2"], dtype=np.float32),
        np.asarray(inputs["b0"], dtype=np.float32),
        np.asarray(inputs["b1"], dtype=np.float32),
        np.asarray(inputs["b2"], dtype=np.float32),
    )
    aev_flat = aev.reshape(-1, K0)
    in_maps = []
    for c in range(N_CORES):
        idx_c = idx[:, c * A_SP : (c + 1) * A_SP]                # [S, A_SP]
        x = aev_flat[idx_c.reshape(-1)].reshape(S, A_SP, K0)     # [S, A_SP, 384]
        xk = x.transpose(0, 2, 1)                                # [S, 384, A_SP]
        xhi = _q8(xk).astype(np.float32)
        xq = np.zeros((S, 128, 2, 2, A_SP), dtype=np.float32)
        selx = [[(xhi, 0), (xhi, 1)], [(xhi, 2), (xhi, 0)]]
        for p in range(2):
            for pl in range(2):
                arr, kt = selx[p][pl]
                xq[:, :, p, pl, :] = arr[:, kt * 128:(kt + 1) * 128, :]
        in_maps.append({"xt": _q8(xq), **shared})

    res = run_bass_kernel_spmd(
        nc, in_maps, core_ids=list(range(N_CORES)), trace=trace, tmpdir=tmpdir
    )

    # host-side tail.  rs holds row-sums of h2 = elu(u2) in the merged-row
    # layout (rescaled units); per-atom E = a*w3 . h2 + b3, so
    #   total = a*sum(rs * w3rep) + (N/S)*sum(b3),  out = total / E
    w3m = np.zeros((128, S, NQ, 3), dtype=np.float64)
    for s in range(S):
        for q in range(NQ):
            for b in range(3):
                for (mi, lo, hi, row) in _L2_PIECES[b]:
                    w3m[row : row + hi - lo, s, q, b] = w3[s, 4 * q + mi, lo:hi, 0]
    w3rep = np.repeat(
        w3m.reshape(128, S * NQ * 3)[:, :, None], NCH, axis=2
    ).reshape(128, S * NQ * 3 * NCH)
    total = 0.0
    for c in range(N_CORES):
        total += ALPHA * float(
            (res.results[c]["rs"].astype(np.float64) * w3rep).sum()
        )
    total += float(b3.astype(np.float64).sum()) * (N_ATOMS // S)
    out = np.array([total / E], dtype=np.float32)
    return out, res


def kernel(**inputs):
    out, _ = _run(inputs, trace=bool(int(os.environ.get("BASS_KERNEL_TRACE", "0"))))
    return out



# revision 14
# speedup vs baseline: 1.2438x; 1.0288x over previous
"""Trainium2 Bass kernel for nn_BmmEnsemble (ANI-style per-species ensemble MLP).

Math (see reference): for each species s (4) and ensemble member e (8), the
species' atoms' AEV rows go through a 384->160->128->96->1 MLP with CELU(0.1)
after the first three layers; the output energy is the global sum over all
atoms of the ensemble-mean of the final scalar.

Key transformation: celu(z, a) = a*elu(z/a), so the whole network is rescaled
so the activation becomes elu (alpha=1) exactly: W0 <- W0/a, biases <- b/a,
w3 <- a*w3, activations h_hat = h/a.  On top of that, layers 0/1 store
g = elu(u) + 1 (>= 0) and fold the "-1" into the next layer's bias via
b_next <- b_next - colsum(W_next).

Layers 0/1 evaluate elu(u)+1 with a SINGLE elementwise pass per tile, split
across two engines to break the baseline's VectorE bottleneck:

 - most tiles: one custom-DVE pass
       elu(u)+1 ~= max(u + 1, clamp(1 + k*u, 0, 1)^4)       (k = 0.21)
   using (1+k*u)^4 ~= e^u and Bernoulli ((1+ku)^4 >= 1+u on the clamp
   range), so the max IS the exact relu branch for u>=0.  The body is
   exactly 8 DVE ALU stages: fma, relu, minn, sq, sq, add, maxx.
 - half the L0a tiles (SCAL_MEMBERS) use a pure-ScalarE path instead:
   rho = Relu(-u), m = Exp(-rho), r = Relu(u) (three ACT passes, exact:
   elu(u)+1 = r + m), and the layer-1 matmul consumes r and m as two
   accumulating rhs.  This moves ~80us of elementwise work to the
   otherwise-idle ScalarE.

Layer 2 keeps the baseline's exact two-pass form (ScalarE Exp + DVE blend
with fused row-sum accum) with alpha=1 semantics.  Layer-0 matmuls run in
bf16 (x, w0 both bf16 - mixed bf16 x f32r is rejected by the BIR verifier):
same PE rate, but FWL halves LDWEIGHTS time and input DMA.  Layers 1/2 stay
f32r because DVE bf16 writes are slower (RMW).  Measured end-to-end error:
6e-4 relative (gate 2e-2).

Distribution: data-parallel over atoms (2048 atoms/species/core,
feature-major), per-species ensemble weights replicated, host applies the
tiny w3 dot and sums the per-core row-sum outputs (the "all-reduce").

Measured on 8 axon-tunneled trn2 cores: ~266us HW exec (baseline with
2-pass celu on every tile: ~294us).  Engine busy: VectorE ~252us (320
passes), ScalarE ~199us (289 ACT passes), TensorE ~211us streaming.
Tried and reverted: chunk-pair [128,2,512] DVE/ACT batching cut VectorE
busy to 220us but single-buffered PSUM pairs (8-bank limit) added ~100us
of dependency stalls (315-367us spans); fp8 fails accuracy (5e-2).
"""

import os
from operator import add as _operator_add

import numpy as np

import concourse.dve_ops as _dve_ops
import concourse.mybir as mybir
import concourse.tile as tile
from concourse import bacc
from concourse.bass_utils import run_bass_kernel_spmd
from concourse.dve_spec import (
    C0,
    C1,
    C2,
    One,
    Spec,
    Src0,
    Src1,
    Zero,
    _has_src1,
    lower,
    maxx,
    minn,
    relu,
    sq,
)
from concourse.dve_uop import DveOpSpec

# ---------------------------------------------------------------- constants
S, E = 4, 8
N_ATOMS = 65536
N_CORES = 8
A_SP = N_ATOMS // S // N_CORES      # atoms per species per core = 2048
CHUNK = 512
NCH = A_SP // CHUNK                 # 4 chunks
K0, H0, H1, H2 = 384, 160, 128, 96
KT = K0 // 128                      # 3 K-tiles for layer 0
NQ = 2                              # member quads per species (E/4)
ALPHA = 0.1
KP = 0.21                           # (1 + KP*u)^4 ~ e^u  (layers 0/1)

F32 = mybir.dt.float32
F32R = mybir.dt.float32r
BF16 = mybir.dt.bfloat16
EXP = mybir.ActivationFunctionType.Exp
RELU = mybir.ActivationFunctionType.Relu

# L0a tiles of these members use the pure-ScalarE path (rho/m/r ACT passes,
# consumer matmul takes r and m as two accumulating rhs) instead of the DVE
# poly pass — balances VectorE vs the otherwise-idle ScalarE.
SCAL_MEMBERS = (0, 2, 4, 6)

# ------------------------------------------------------- custom DVE ops
# POLY_ELU4: out = max(z + C1, clamp(z*C2 + C0, 0, 1)^4)  ==  elu(u)+1 approx
# with u = z + b;  C0 = 1 + k*b (per-partition), C1 = b + 1, C2 = k.
_B_POLY = maxx(Src0 + C1, sq(sq(minn(relu(Src0 * C2 + C0), One))))
# CELU blend (exact, for layer 2 with alpha=1): in1 = exp(u) from ScalarE;
# out = relu(z + C0) + min(in1*C1 - C1, 0) = elu(u) for C1 = 1.
_B_BLEND = relu(Src0 + C0) + minn(Src1 * C1 - C1, Zero)


def _ref_poly(in0, in1, s0, s1, imm2):
    z = in0.astype(np.float32)
    s = np.minimum(np.maximum(z * imm2 + s0, 0.0), 1.0)
    return np.maximum(z + s1, (s * s) * (s * s)).astype(np.float32)


def _ref_blend_acc(in0, in1, s0, s1, imm2):
    z = in0.astype(np.float32) + s0
    b = (np.maximum(z, 0.0)
         + np.minimum(in1.astype(np.float32) * s1 - s1, 0.0)).astype(np.float32)
    return b, b.reshape(b.shape[0], -1).sum(axis=-1, keepdims=True)


def _mk_op(name, spec):
    row = _dve_ops._CUSTOM_DVE_ROW_BASE + len(_dve_ops.OPS)
    assert row < 0x20, "custom-DVE opcode rows exhausted"
    _dve_ops._SUB_OPCODE_FOR_NAME[name] = row
    shas = {}
    for ver in ("v3", "v4"):
        s = DveOpSpec(
            name=name, opcode=row, uops=lower(spec, ver=ver), rd1_en=_has_src1(spec)
        )
        shas[ver] = s.sha(ver)
    op = _dve_ops.DveOp(name, spec, subdim=False, uops_sha=shas)
    _dve_ops.OPS.append(op)
    _dve_ops.CUSTOM_DVE_SPECS[name] = spec
    return op


def _register_ops():
    existing = {o.name: o for o in _dve_ops.OPS}
    if "POLY_ELU4_ANT" in existing:
        return existing["POLY_ELU4_ANT"], existing["ELU_BLEND_ACC_ANT"]
    poly = _mk_op("POLY_ELU4_ANT", Spec(body=_B_POLY, reference=_ref_poly))
    blend = _mk_op(
        "ELU_BLEND_ACC_ANT",
        Spec(body=_B_BLEND, accum=_operator_add, accum_init=Zero,
             reference=_ref_blend_acc),
    )
    return poly, blend


# ------------------------------------------------------------ device build
_NC = None


def _build_nc():
    global _NC
    if _NC is not None:
        return _NC
    POLY, BLEND_ACC = _register_ops()

    nc = bacc.Bacc("TRN2", target_bir_lowering=False, debug=False)

    # per-core inputs (bf16 feature-major atoms)
    xt_d = nc.dram_tensor("xt", [S, KT, 128, A_SP], BF16, kind="ExternalInput")
    # replicated weight packs (bf16, rescaled: w0 includes the 1/alpha).
    w0a_d = nc.dram_tensor("w0a", [S, KT, 128, E * 128], BF16, kind="ExternalInput")
    w0b_d = nc.dram_tensor("w0b4", [S, KT, 128, NQ * 128], BF16, kind="ExternalInput")
    w1a_d = nc.dram_tensor("w1a", [S, 128, E * H1], F32R, kind="ExternalInput")
    w1b_d = nc.dram_tensor("w1b", [S, 128, E * H1], F32R, kind="ExternalInput")
    w2_d = nc.dram_tensor("w2p", [S, 128, NQ * 6 * 128], F32R, kind="ExternalInput")
    # bias packs; *_c0 = 1 + k*b (poly clamp offset), *_c1 = b + 1 (linear).
    b0a_c0 = nc.dram_tensor("b0a_c0", [128, S * E], F32, kind="ExternalInput")
    b0a_c1 = nc.dram_tensor("b0a_c1", [128, S * E], F32, kind="ExternalInput")
    b0b_c0 = nc.dram_tensor("b0b_c0", [128, S * NQ], F32, kind="ExternalInput")
    b0b_c1 = nc.dram_tensor("b0b_c1", [128, S * NQ], F32, kind="ExternalInput")
    b1_c0 = nc.dram_tensor("b1_c0", [H1, S * E], F32, kind="ExternalInput")
    b1_c1 = nc.dram_tensor("b1_c1", [H1, S * E], F32, kind="ExternalInput")
    # ScalarE-path packs: plain bias and negated bias
    b0a_b = nc.dram_tensor("b0a_b", [128, S * E], F32, kind="ExternalInput")
    b0a_nb = nc.dram_tensor("b0a_nb", [128, S * E], F32, kind="ExternalInput")
    # L2 (exact 2-pass): single bias pack b2 (ACT bias and blend s0)
    b2_d = nc.dram_tensor("b2m_d", [128, S * NQ * 3], F32, kind="ExternalInput")
    # output: per-(s,quad,bank,chunk) row-sums of h2 = elu(u2) (merged rows)
    rs_d = nc.dram_tensor("rs", [128, S * NQ * 3 * NCH], F32, kind="ExternalOutput")

    with tile.TileContext(nc) as tc:
        with (
            tc.tile_pool(name="xp", bufs=2) as xp,
            tc.tile_pool(name="w0pool", bufs=2) as w0p,
            tc.tile_pool(name="w1pool", bufs=2) as w1p,
            tc.tile_pool(name="bp", bufs=1) as bp,
            tc.tile_pool(name="ep", bufs=4) as ep,
            tc.tile_pool(name="gp", bufs=6) as gp,
            tc.tile_pool(name="ps", bufs=2, space="PSUM") as psp,
        ):
            # warm the ACT Exp table during the initial DMA wait
            warm = bp.tile([1, 1], F32, tag="warm", name="warm")
            nc.vector.memset(warm[:], 0.0)
            nc.scalar.activation(warm[:], warm[:], EXP)

            B = {}
            _bias_dmas = []
            for nm, d, p in (
                ("b0a_c0", b0a_c0, 128), ("b0a_c1", b0a_c1, 128),
                ("b0b_c0", b0b_c0, 128), ("b0b_c1", b0b_c1, 128),
                ("b1_c0", b1_c0, H1), ("b1_c1", b1_c1, H1),
                ("b0a_b", b0a_b, 128), ("b0a_nb", b0a_nb, 128),
                ("b2_d", b2_d, 128),
            ):
                t = bp.tile([p, d.shape[-1]], F32, tag=nm, name=nm)
                _bias_dmas.append((t, d))
                B[nm] = t
            RS = bp.tile([128, S * NQ * 3 * NCH], F32, tag="RS", name="RS")

            for s in range(S):
                xk = []
                w0ak = []
                w0bk = []
                # first-chunk x slices + all weights first, so chunk-0 compute
                # starts as early as possible; remaining x chunks stream after
                for k in range(KT):
                    xt = xp.tile([128, A_SP], BF16, tag=f"x{k}", name=f"x_{s}_{k}")
                    nc.sync.dma_start(xt[:, 0:CHUNK], xt_d[s, k, :, 0:CHUNK])
                    xk.append(xt)
                for k in range(KT):
                    wt = w0p.tile([128, E * 128], BF16, tag=f"w0a{k}", name=f"w0a_{s}_{k}")
                    nc.sync.dma_start(wt[:], w0a_d[s, k])
                    w0ak.append(wt)
                    wbt = w0p.tile([128, NQ * 128], BF16, tag=f"w0b{k}", name=f"w0b_{s}_{k}")
                    nc.sync.dma_start(wbt[:], w0b_d[s, k])
                    w0bk.append(wbt)
                if s == 0:
                    # bias packs are tiny (~130KB) and needed by the very
                    # first poly pass (~8us in) - emit them before the bulky
                    # w1/w2 transfers so the first quad's elementwise work
                    # isn't stalled behind 3.5MB of layer-1/2 weights.
                    for t, d in _bias_dmas:
                        nc.sync.dma_start(t[:], d[:])
                w1at = w1p.tile([128, E * H1], F32R, tag="w1a", name=f"w1a_{s}")
                nc.sync.dma_start(w1at[:], w1a_d[s])
                w1bt = w1p.tile([128, E * H1], F32R, tag="w1b", name=f"w1b_{s}")
                nc.sync.dma_start(w1bt[:], w1b_d[s])
                w2t = w1p.tile([128, NQ * 6 * 128], F32R, tag="w2", name=f"w2_{s}")
                nc.sync.dma_start(w2t[:], w2_d[s])
                for k in range(KT):
                    nc.sync.dma_start(
                        xk[k][:, CHUNK:A_SP], xt_d[s, k, :, CHUNK:A_SP]
                    )

                for c in range(NCH):
                    cs = slice(c * CHUNK, (c + 1) * CHUNK)
                    for q in range(NQ):
                        sq_i = s * NQ + q
                        # ---- merged layer-0b for the 4 members of this quad
                        ps0b = psp.tile([128, CHUNK], F32, tag="l0b", bufs=1)
                        for k in range(KT):
                            nc.tensor.matmul(
                                ps0b[:],
                                w0bk[k][:, q * 128 : (q + 1) * 128],
                                xk[k][:, cs],
                                start=(k == 0),
                                stop=(k == KT - 1),
                            )
                        g0b = gp.tile([128, CHUNK], F32R, tag="g0b")
                        nc.vector._custom_dve(
                            POLY, out=g0b[:], in0=ps0b[:],
                            s0=B["b0b_c0"][:, sq_i : sq_i + 1],
                            s1=B["b0b_c1"][:, sq_i : sq_i + 1], imm2=KP,
                        )

                        def do_l2_bank(b):
                            # merged layer 2, bank b of the quad (2 zero-padded
                            # matmuls); exact elu via ScalarE Exp + DVE blend
                            # with fused row-sum accum into RS.
                            (m0, m1) = ((0, 1), (1, 2), (2, 3))[b]
                            ps2 = psp.tile([128, CHUNK], F32, tag="l2", name=f"ps2_{b}")
                            off = (q * 3 + b) * 2 * 128
                            nc.tensor.matmul(
                                ps2[:], w2t[:, off : off + 128], g1s[m0][:],
                                start=True, stop=False,
                            )
                            nc.tensor.matmul(
                                ps2[:], w2t[:, off + 128 : off + 256], g1s[m1][:],
                                start=False, stop=True,
                            )
                            sqb = (s * NQ + q) * 3 + b
                            e2 = ep.tile([128, CHUNK], F32, tag="e2", name=f"e2_{b}")
                            nc.scalar.activation(
                                e2[:], ps2[:], EXP,
                                bias=B["b2_d"][:, sqb : sqb + 1], scale=1.0,
                            )
                            g2 = gp.tile([128, CHUNK], F32, tag="g2", name=f"g2_{b}")
                            nc.vector._custom_dve(
                                BLEND_ACC, out=g2[:],
                                accum_out=RS[:, sqb * NCH + c : sqb * NCH + c + 1],
                                in0=ps2[:], in1=e2[:],
                                s0=B["b2_d"][:, sqb : sqb + 1], s1=1.0,
                            )

                        g1s = []
                        for e in range(q * 4, q * 4 + 4):
                            se = s * E + e
                            # ---- layer 0a (first 128 features of member e)
                            ps0a = psp.tile([128, CHUNK], F32, tag="l0a", bufs=3)
                            for k in range(KT):
                                nc.tensor.matmul(
                                    ps0a[:],
                                    w0ak[k][:, e * 128 : (e + 1) * 128],
                                    xk[k][:, cs],
                                    start=(k == 0),
                                    stop=(k == KT - 1),
                                )
                            ps1 = psp.tile([H1, CHUNK], F32, tag="l1", bufs=2)
                            if e in SCAL_MEMBERS:
                                # pure-ScalarE path: g0a = r + m exactly
                                # (elu+1 = relu(u) + exp(-relu(-u))); the L1
                                # matmul consumes r and m as two rhs.
                                rho = ep.tile([128, CHUNK], F32, tag="rho")
                                nc.scalar.activation(
                                    rho[:], ps0a[:], RELU,
                                    bias=B["b0a_nb"][:, se : se + 1], scale=-1.0,
                                )
                                m0 = gp.tile([128, CHUNK], F32R, tag="m0")
                                nc.scalar.activation(
                                    m0[:], rho[:], EXP, scale=-1.0,
                                )
                                r0 = gp.tile([128, CHUNK], F32R, tag="r0")
                                nc.scalar.activation(
                                    r0[:], ps0a[:], RELU,
                                    bias=B["b0a_b"][:, se : se + 1], scale=1.0,
                                )
                                nc.tensor.matmul(
                                    ps1[:], w1at[:, e * H1 : (e + 1) * H1], r0[:],
                                    start=True, stop=False,
                                )
                                nc.tensor.matmul(
                                    ps1[:], w1at[:, e * H1 : (e + 1) * H1], m0[:],
                                    start=False, stop=False,
                                )
                            else:
                                g0a = gp.tile([128, CHUNK], F32R, tag="g0a")
                                nc.vector._custom_dve(
                                    POLY, out=g0a[:], in0=ps0a[:],
                                    s0=B["b0a_c0"][:, se : se + 1],
                                    s1=B["b0a_c1"][:, se : se + 1], imm2=KP,
                                )
                                nc.tensor.matmul(
                                    ps1[:], w1at[:, e * H1 : (e + 1) * H1], g0a[:],
                                    start=True, stop=False,
                                )
                            nc.tensor.matmul(
                                ps1[:], w1bt[:, e * H1 : (e + 1) * H1], g0b[:],
                                start=False, stop=True,
                            )
                            g1 = gp.tile([H1, CHUNK], F32R, tag="g1", bufs=6)
                            nc.vector._custom_dve(
                                POLY, out=g1[:], in0=ps1[:],
                                s0=B["b1_c0"][:, se : se + 1],
                                s1=B["b1_c1"][:, se : se + 1], imm2=KP,
                            )
                            g1s.append(g1)
                            if len(g1s) >= 2:
                                do_l2_bank(len(g1s) - 2)
            nc.sync.dma_start(rs_d[:], RS[:])
    nc.compile()
    _NC = nc
    return nc


# ------------------------------------------------------------- host side
# merged-L2 bank layout: per quad, (bank, piece) -> (member_in_quad,
# w2-col range, psum-row offset)
_L2_PIECES = [
    [(0, 0, 96, 0), (1, 0, 32, 96)],
    [(1, 32, 96, 0), (2, 0, 64, 64)],
    [(2, 64, 96, 0), (3, 0, 96, 32)],
]


def _prep_shared(w0, w1, w2, b0, b1, b2):
    """Pack rescaled weights/biases into the device layouts.

    Rescaling: W0/alpha, biases b/alpha; layer-0/1 activations stored as
    g = elu+1, so layer 1/2 effective bias is b/alpha - colsum(W).
    """
    f = np.float32
    w0 = (w0.astype(np.float64) / ALPHA)
    b0e = (b0[:, :, 0, :].astype(np.float64) / ALPHA)                 # [S,E,160]
    b1e = b1[:, :, 0, :].astype(np.float64) / ALPHA - w1.astype(np.float64).sum(2)
    b2e = b2[:, :, 0, :].astype(np.float64) / ALPHA - w2.astype(np.float64).sum(2)

    w0r = w0.reshape(S, E, KT, 128, H0)
    w0a = np.ascontiguousarray(
        w0r[..., :128].transpose(0, 2, 3, 1, 4).reshape(S, KT, 128, E * 128)
    ).astype(f)
    w0b4 = np.ascontiguousarray(
        w0r[..., 128:].transpose(0, 2, 3, 1, 4).reshape(S, KT, 128, E * (H0 - 128))
    ).astype(f)
    w1a = np.ascontiguousarray(
        w1[:, :, :128, :].transpose(0, 2, 1, 3).reshape(S, 128, E * H1)
    ).astype(f)
    w1b = np.zeros((S, 4, 32, E, H1), dtype=f)
    for e in range(E):
        w1b[:, e % 4, :, e, :] = w1[:, e, 128:, :]
    w1b = np.ascontiguousarray(w1b.reshape(S, 128, E * H1))
    # merged-L2 packs
    w2pk = np.zeros((S, NQ, 3, 2, 128, 128), dtype=f)  # [s,q,b,piece,K,M]
    b2m = np.zeros((S, NQ, 3, 128), dtype=np.float64)
    for s in range(S):
        for q in range(NQ):
            for b in range(3):
                for piece, (mi, lo, hi, row) in enumerate(_L2_PIECES[b]):
                    e = 4 * q + mi
                    w2pk[s, q, b, piece, :, row : row + hi - lo] = w2[s, e, :, lo:hi]
                    b2m[s, q, b, row : row + hi - lo] = b2e[s, e, lo:hi]
    w2p = np.ascontiguousarray(
        w2pk.transpose(0, 4, 1, 2, 3, 5).reshape(S, 128, NQ * 6 * 128)
    )

    def col_pack(b, lo, hi):
        # b [S,E,P] -> [hi-lo, S*E]
        return np.ascontiguousarray(
            b[:, :, lo:hi].reshape(S * E, hi - lo).T
        ).astype(np.float64)

    b0a_c1 = col_pack(b0e, 0, 128)
    b0b_c1 = np.ascontiguousarray(
        b0e[:, :, 128:].reshape(S, NQ, 4 * (H0 - 128)).transpose(2, 0, 1).reshape(128, S * NQ)
    )
    b1_c1 = col_pack(b1e, 0, H1)
    b2m_d = np.ascontiguousarray(b2m.reshape(S * NQ * 3, 128).T).astype(f)

    shared = {
        "w0a": w0a, "w0b4": w0b4, "w1a": w1a, "w1b": w1b, "w2p": w2p,
        "b2m_d": b2m_d,
    }
    for nm, b in (("b0a", b0a_c1), ("b0b", b0b_c1), ("b1", b1_c1)):
        shared[f"{nm}_c0"] = (1.0 + KP * b).astype(f)
        shared[f"{nm}_c1"] = (b + 1.0).astype(f)
    shared["b0a_b"] = b0a_c1.astype(f)
    shared["b0a_nb"] = (-b0a_c1).astype(f)
    return shared


def _run(inputs, trace=False, tmpdir=None):
    import ml_dtypes

    aev = np.asarray(inputs["aev"], dtype=np.float32)
    idx = np.asarray(inputs["idx"], dtype=np.int32)
    w3 = np.asarray(inputs["w3"], dtype=np.float32)
    b3 = np.asarray(inputs["b3"], dtype=np.float32)

    nc = _build_nc()
    shared = _prep_shared(
        np.asarray(inputs["w0"], dtype=np.float32),
        np.asarray(inputs["w1"], dtype=np.float32),
        np.asarray(inputs["w2"], dtype=np.float32),
        np.asarray(inputs["b0"], dtype=np.float32),
        np.asarray(inputs["b1"], dtype=np.float32),
        np.asarray(inputs["b2"], dtype=np.float32),
    )
    bf = ml_dtypes.bfloat16
    shared["w0a"] = shared["w0a"].astype(bf)
    shared["w0b4"] = shared["w0b4"].astype(bf)

    aev_flat = aev.reshape(-1, K0)
    in_maps = []
    for c in range(N_CORES):
        idx_c = idx[:, c * A_SP : (c + 1) * A_SP]                # [S, A_SP]
        x = aev_flat[idx_c.reshape(-1)].reshape(S, A_SP, K0)     # [S, A_SP, 384]
        xt = np.ascontiguousarray(x.transpose(0, 2, 1)).reshape(S, KT, 128, A_SP)
        in_maps.append({"xt": xt.astype(bf), **shared})

    res = run_bass_kernel_spmd(
        nc, in_maps, core_ids=list(range(N_CORES)), trace=trace, tmpdir=tmpdir
    )

    # host-side tail.  rs holds row-sums of h2 = elu(u2) in the merged-row
    # layout (rescaled units); per-atom E = a*w3 . h2 + b3, so
    #   total = a*sum(rs * w3rep) + (N/S)*sum(b3),  out = total / E
    w3m = np.zeros((128, S, NQ, 3), dtype=np.float64)
    for s in range(S):
        for q in range(NQ):
            for b in range(3):
                for (mi, lo, hi, row) in _L2_PIECES[b]:
                    w3m[row : row + hi - lo, s, q, b] = w3[s, 4 * q + mi, lo:hi, 0]
    w3rep = np.repeat(
        w3m.reshape(128, S * NQ * 3)[:, :, None], NCH, axis=2
    ).reshape(128, S * NQ * 3 * NCH)
    total = 0.0
    for c in range(N_CORES):
        total += ALPHA * float(
            (res.results[c]["rs"].astype(np.float64) * w3rep).sum()
        )
    total += float(b3.astype(np.float64).sum()) * (N_ATOMS // S)
    out = np.array([total / E], dtype=np.float32)
    return out, res


def kernel(**inputs):
    out, _ = _run(inputs, trace=bool(int(os.environ.get("BASS_KERNEL_TRACE", "0"))))
    return out

